# revision 1
# baseline (speedup 1.0000x reference)
"""Trainium2 Bass kernel for the DGL-style ChebConv GNN classifier.

Strategy (8 NeuronCores, SPMD):
  - Nodes sharded contiguously across cores (12.5K rows each); edges owned by
    the core that owns their dst.
  - Per ChebConv layer, the Laplacian application ahat() is computed twice
    (Chebyshev K=3) as: gather src rows from a replicated node table
    (dma_gather, int16 indices over 32K-row chunks), then segment-sum via
    one-hot matmuls accumulating in PSUM, evacuated with per-partition norm
    scalings.  Work happens in "Y = X * norm" space so every scaling is a
    per-dst-row (per-partition) tensor_scalar.
  - Node tables are re-replicated between passes with AllGather collectives.
  - The dense (concat @ W) matmuls consume PE-transposed feature-major
    blocks; relu(+scale) epilogue on the scalar engine writes the next
    layer's table shard.
  - Readout: per-core one-hot (graph id) matmul partial sums + AllReduce,
    then the small MLP classifier on-chip.

kernel(**inputs) takes FULL unsharded inputs and returns the FULL [G, 10]
output; all sharding happens inside.
"""

import math
import os

import numpy as np

import concourse.bass as bass
import concourse.bacc as bacc
import concourse.mybir as mybir
import concourse.tile as tile
from concourse.bass_utils import run_bass_kernel_spmd

NCORES = 8
P = 128
CHUNK = 32768          # int16 index range for dma_gather
BB = 4                 # dst blocks per batch (bounds live scatter-psum banks)
MAX_CALL = 1024        # max slots per dma_gather call (SWDGE carveout = 1024 descs)
F32 = mybir.dt.float32
I16 = mybir.dt.int16


def _wrap16(local_idx):
    """[L] -> [128, L/16]: element i at [i%16, i//16], replicated to 128
    partitions (8 Q7 cores each read a 16-partition group)."""
    L = local_idx.shape[0]
    w = local_idx.reshape(L // 16, 16).T.copy()
    return np.tile(w, (8, 1))


def _preprocess(src, dst, graph_ids, N, G):
    """Build the shared (SPMD-equal) program structure + per-core data."""
    E = src.shape[0]
    NLOC = (N + NCORES - 1) // NCORES
    NB = (NLOC + P - 1) // P            # dst blocks per core
    NBATCH = (NB + BB - 1) // BB
    NCH = (N + CHUNK - 1) // CHUNK

    deg = np.bincount(dst, minlength=N).astype(np.float32)
    norm = np.clip(deg, 1.0, None) ** -0.5          # [N]
    norm2 = norm * norm
    inv_norm = 1.0 / norm

    # ---- per-core edge streams ----------------------------------------
    core_of = dst // NLOC
    per_core = []
    counts = np.zeros((NCORES, NBATCH, NCH), dtype=np.int64)
    for c in range(NCORES):
        m = core_of == c
        s = src[m]
        dl = dst[m] - c * NLOC
        blk = dl // P
        bat = blk // BB
        ch = s // CHUNK
        order = np.lexsort((dl, ch, bat))
        s, dl, bat, ch = s[order], dl[order], bat[order], ch[order]
        key = bat * NCH + ch
        cnt = np.bincount(key, minlength=NBATCH * NCH).reshape(NBATCH, NCH)
        counts[c] = cnt
        per_core.append((s, dl, key))

    # equalized run lengths (128-aligned), shared across cores
    runlen = (
        ((counts.max(axis=0) + P - 1) // P) * P
    )  # [NBATCH, NCH]
    run_off = np.zeros((NBATCH, NCH), dtype=np.int64)
    tot = 0
    for t in range(NBATCH):
        for ch in range(NCH):
            run_off[t, ch] = tot
            tot += runlen[t, ch]
    TOT = int(tot)
    NSUB = TOT // P

    # ---- slot arrays per core -----------------------------------------
    slot_src = np.empty((NCORES, TOT), dtype=np.int64)
    slot_dstl = np.empty((NCORES, TOT), dtype=np.int64)
    for c in range(NCORES):
        s, dl, key = per_core[c]
        ssrc = np.empty(TOT, dtype=np.int64)
        sdst = np.full(TOT, -1, dtype=np.int64)
        # fill pads with the chunk's base row (valid gather, zero one-hot)
        for t in range(NBATCH):
            for ch in range(NCH):
                o, L = run_off[t, ch], runlen[t, ch]
                ssrc[o : o + L] = ch * CHUNK if ch * CHUNK < N else 0
        pos = np.empty(len(key), dtype=np.int64)
        # edges are sorted by key; place each run's edges at its offset
        kcnt = counts[c].reshape(-1)
        koff = run_off.reshape(-1)
        start = 0
        for k in range(NBATCH * NCH):
            n = kcnt[k]
            pos[start : start + n] = koff[k] + np.arange(n)
            start += n
        ssrc[pos] = s
        sdst[pos] = dl
        slot_src[c] = ssrc
        slot_dstl[c] = sdst

    # ---- gather calls (shared structure) ------------------------------
    # each call: one (batch, chunk) run split into <=MAX_CALL slot segments
    calls = []  # (chunk, slot_off, length)
    for t in range(NBATCH):
        for ch in range(NCH):
            o, L = int(run_off[t, ch]), int(runlen[t, ch])
            while L > 0:
                seg = min(L, MAX_CALL)
                calls.append((ch, o, seg))
                o += seg
                L -= seg

    # idx16_all: [128, TOT/16] int16 per core, per-call wrap
    idx16 = np.zeros((NCORES, P, TOT // 16), dtype=np.int16)
    for c in range(NCORES):
        for ch, o, L in calls:
            local = (slot_src[c, o : o + L] - ch * CHUNK).astype(np.int16)
            idx16[c][:, o // 16 : (o + L) // 16] = _wrap16(local)

    # map subtile -> (call index, column within call)
    sub_call = np.empty(NSUB, dtype=np.int64)
    sub_col = np.empty(NSUB, dtype=np.int64)
    for k, (ch, o, L) in enumerate(calls):
        for j in range(L // P):
            sub_call[o // P + j] = k
            sub_col[o // P + j] = j

    # ---- (subtile, block) pairs: union across cores -------------------
    blk_all = slot_dstl // P  # [NCORES, TOT], -1 for pads
    pairs = []  # (subtile, block)
    for sidx in range(NSUB):
        sl = blk_all[:, sidx * P : (sidx + 1) * P]
        present = np.unique(sl[sl >= 0])
        for b in present:
            pairs.append((sidx, int(b)))
    NPAIRS = len(pairs)

    # per-block pair index ranges (first/last occurrence in pair order)
    first_pair = {}
    last_pair = {}
    for j, (sidx, b) in enumerate(pairs):
        if b not in first_pair:
            first_pair[b] = j
        last_pair[b] = j

    # dstsel: [128, NPAIRS] fp32 per core
    dstsel = np.full((NCORES, P, NPAIRS), -1.0, dtype=np.float32)
    for j, (sidx, b) in enumerate(pairs):
        sl = slot_dstl[:, sidx * P : (sidx + 1) * P]  # [NCORES, 128]
        m = (sl // P) == b
        col = np.where(m, (sl - b * P).astype(np.float32), -1.0)
        dstsel[:, :, j] = col

    # ---- per-block norm columns & graph sel ---------------------------
    # normc[c]: [128, 4*NB]: kinds (0: -norm2, 1: -2*norm2, 2: inv_norm, 3: norm)
    normc = np.zeros((NCORES, P, 4 * NB), dtype=np.float32)
    gsel = np.full((NCORES, P, NB), -1.0, dtype=np.float32)
    for c in range(NCORES):
        lo = c * NLOC
        hi = min(lo + NLOC, N)
        n = hi - lo
        pad = NB * P - n
        nn = np.pad(norm[lo:hi], (0, pad)).reshape(NB, P).T
        n2 = np.pad(norm2[lo:hi], (0, pad)).reshape(NB, P).T
        iv = np.pad(inv_norm[lo:hi], (0, pad)).reshape(NB, P).T
        normc[c][:, 0 * NB : 1 * NB] = -n2
        normc[c][:, 1 * NB : 2 * NB] = -2.0 * n2
        normc[c][:, 2 * NB : 3 * NB] = iv
        normc[c][:, 3 * NB : 4 * NB] = nn
        gs = np.pad(graph_ids[lo:hi].astype(np.float32), (0, pad), constant_values=-1.0)
        gsel[c] = gs.reshape(NB, P).T

    block_rows = [min(P, NLOC - b * P) for b in range(NB)]

    return dict(
        N=N, E=E, G=G, NLOC=NLOC, NB=NB, NBATCH=NBATCH, NCH=NCH,
        TOT=TOT, NSUB=NSUB, calls=calls, pairs=pairs,
        first_pair=first_pair, last_pair=last_pair,
        sub_call=sub_call, sub_col=sub_col,
        idx16=idx16, dstsel=dstsel, normc=normc, gsel=gsel,
        block_rows=block_rows, norm=norm,
    )


DIN = [128, 128, 128, 256]
DOUT = [128, 128, 256, 512]


def _build(S):
    """Build the SPMD Bass program (shared across cores)."""
    KSTAGE = int(os.environ.get("KSTAGE", "99"))
    KSINGLE = os.environ.get("KSINGLE", "0") == "1"
    NLOC, NB, NCH, TOT = S["NLOC"], S["NB"], S["NCH"], S["TOT"]
    NPAIRS = len(S["pairs"])
    NTAB = NCORES * NLOC  # table rows (>= N)

    nc = bacc.Bacc(trn_type="TRN2", num_devices=1 if KSINGLE else NCORES,
                   dynamic_dma_scratch_size=32768, num_swdge_queues=4)

    sig_in = nc.dram_tensor("sig", [NLOC, 128], F32, kind="ExternalInput")
    idx_in = nc.dram_tensor("idx16", [P, TOT // 16], I16, kind="ExternalInput")
    dsel_in = nc.dram_tensor("dstsel", [P, NPAIRS], F32, kind="ExternalInput")
    normc_in = nc.dram_tensor("normc", [P, 4 * NB], F32, kind="ExternalInput")
    gsel_in = nc.dram_tensor("gsel", [P, NB], F32, kind="ExternalInput")
    iota_in = nc.dram_tensor("iota", [P, P], F32, kind="ExternalInput")
    ident_in = nc.dram_tensor("ident", [P, P], F32, kind="ExternalInput")
    w_in = [
        nc.dram_tensor(f"W{l}", [3 * DIN[l], DOUT[l]], F32, kind="ExternalInput")
        for l in range(4)
    ]
    bt_in = [
        nc.dram_tensor(f"Bt{l}", [P, DOUT[l]], F32, kind="ExternalInput")
        for l in range(4)
    ]
    wm1_in = nc.dram_tensor("Wm1", [512, 512], F32, kind="ExternalInput")
    bm1_in = nc.dram_tensor("Bm1", [P, 512], F32, kind="ExternalInput")
    wm2_in = nc.dram_tensor("Wm2", [512, 16], F32, kind="ExternalInput")
    bm2_in = nc.dram_tensor("Bm2", [P, 16], F32, kind="ExternalInput")
    out = nc.dram_tensor("out", [P, 16], F32, kind="ExternalOutput")

    with tile.TileContext(nc) as tc:
        with (
            tc.tile_pool(name="dram", bufs=1, space="DRAM") as dram,
            tc.tile_pool(name="res", bufs=1) as res,
            tc.tile_pool(name="sb", bufs=3) as sb,
            tc.tile_pool(name="scp", bufs=1, space="PSUM") as scp,
            tc.tile_pool(name="pp", bufs=2, space="PSUM") as pp,
            tc.tile_pool(name="tpp", bufs=1, space="PSUM") as tpp,
            tc.tile_pool(name="rdp", bufs=1, space="PSUM") as rdp,
        ):
            # ------- resident metadata -------
            idx_sb = res.tile([P, TOT // 16], I16)
            dsel_sb = res.tile([P, NPAIRS], F32)
            normc_sb = res.tile([P, 4 * NB], F32)
            gsel_sb = res.tile([P, NB], F32)
            iota_sb = res.tile([P, P], F32)
            ident_sb = res.tile([P, P], F32)
            nc.sync.dma_start(out=idx_sb[:], in_=idx_in[:, :])
            nc.sync.dma_start(out=dsel_sb[:], in_=dsel_in[:, :])
            nc.sync.dma_start(out=normc_sb[:], in_=normc_in[:, :])
            nc.sync.dma_start(out=gsel_sb[:], in_=gsel_in[:, :])
            nc.sync.dma_start(out=iota_sb[:], in_=iota_in[:, :])
            nc.sync.dma_start(out=ident_sb[:], in_=ident_in[:, :])
            w_sb = []
            for l in range(4):
                nchk = 3 * DIN[l] // P
                t = res.tile([P, nchk * DOUT[l]], F32, tag=f"W{l}")
                for j in range(nchk):
                    nc.sync.dma_start(
                        out=t[:, j * DOUT[l] : (j + 1) * DOUT[l]],
                        in_=w_in[l][j * P : (j + 1) * P, :],
                    )
                w_sb.append(t)
            bt_sb = []
            for l in range(4):
                t = res.tile([P, DOUT[l]], F32, tag=f"Bt{l}")
                nc.sync.dma_start(out=t[:], in_=bt_in[l][:, :])
                bt_sb.append(t)
            wm1_sb = res.tile([P, 4 * 512], F32)
            for j in range(4):
                nc.sync.dma_start(
                    out=wm1_sb[:, j * 512 : (j + 1) * 512],
                    in_=wm1_in[j * P : (j + 1) * P, :],
                )
            bm1_sb = res.tile([P, 512], F32)
            nc.sync.dma_start(out=bm1_sb[:], in_=bm1_in[:, :])
            wm2_sb = res.tile([P, 4 * 16], F32)
            for j in range(4):
                nc.sync.dma_start(
                    out=wm2_sb[:, j * 16 : (j + 1) * 16],
                    in_=wm2_in[j * P : (j + 1) * P, :],
                )
            bm2_sb = res.tile([P, 16], F32)
            nc.sync.dma_start(out=bm2_sb[:], in_=bm2_in[:, :])

            # ------- DRAM tables -------
            y0s = [dram.tile([NLOC, DIN[l]], F32, tag=f"y0s{l}", name=f"y0s{l}") for l in range(4)]
            y1s = [dram.tile([NLOC, DIN[l]], F32, tag=f"y1s{l}", name=f"y1s{l}") for l in range(4)]
            y0f = [dram.tile([NTAB, DIN[l]], F32, tag=f"y0f{l}", name=f"y0f{l}", addr_space="Shared") for l in range(4)]
            y1f = [dram.tile([NTAB, DIN[l]], F32, tag=f"y1f{l}", name=f"y1f{l}", addr_space="Shared") for l in range(4)]

            RG = [list(range(NCORES))]

            def ag(shard, full):
                if KSINGLE:
                    nc.sync.dma_start(out=full[0 : shard.shape[0], :], in_=shard[:])
                    return
                nc.gpsimd.collective_compute(
                    "AllGather", mybir.AluOpType.bypass,
                    replica_groups=RG, ins=[shard[:]], outs=[full[:]],
                )

            def ncol(kind, b):
                return normc_sb[:, kind * NB + b : kind * NB + b + 1]

            # ------- P0: Y0_0 = signal * norm -------
            for b in range(NB):
                rows = S["block_rows"][b]
                sg = sb.tile([P, 128], F32, tag="sg")
                if rows < P:
                    nc.any.memset(sg[:], 0.0)
                nc.sync.dma_start(out=sg[:rows], in_=sig_in[b * P : b * P + rows, :])
                o = sb.tile([P, 128], F32, tag="p0o")
                nc.any.tensor_scalar(
                    out=o[:], in0=sg[:], scalar1=ncol(3, b), scalar2=None,
                    op0=mybir.AluOpType.mult,
                )
                nc.sync.dma_start(out=y0s[0][b * P : b * P + rows, :], in_=o[:rows])
            ag(y0s[0], y0f[0])

            def dump_and_stop(tab):
                d = sb.tile([P, 16], F32, tag="dmp", name="dmp")
                nc.sync.dma_start(out=d[:], in_=tab[0:P, 0:16])
                nc.sync.dma_start(out=out[:, :], in_=d[:])

            stopped = KSTAGE == 0
            if stopped:
                dump_and_stop(y0f[0])

            # ------- scatter unit -------
            def scatter_unit(table, D, evac_fn):
                g_tiles = {}
                psums = {}
                emitted = -1
                for j, (sidx, b) in enumerate(S["pairs"]):
                    k = int(S["sub_call"][sidx])
                    if k > emitted:
                        for kk in range(emitted + 1, k + 1):
                            ch, o, L = S["calls"][kk]
                            rows_ch = min(CHUNK, NTAB - ch * CHUNK)
                            g = sb.tile([P, (MAX_CALL // P) * 256], F32, tag="g", bufs=4, name="g")
                            nc.gpsimd.dma_gather(
                                out_ap=g[:, : (L // P) * D].rearrange(
                                    "p (k d) -> p k d", d=D
                                ),
                                in_ap=table[ch * CHUNK : ch * CHUNK + rows_ch, :],
                                idxs_ap=idx_sb[:, o // 16 : (o + L) // 16],
                                num_idxs=L,
                                num_idxs_reg=L,
                                elem_size=D,
                                queue_num=kk % 4,
                            )
                            g_tiles[kk] = g
                        emitted = k
                    col = int(S["sub_col"][sidx])
                    s_t = sb.tile([P, P], F32, tag="s", bufs=8, name="s_t")
                    if j % 3 == 0:
                        # ACT path: onehot = Relu(1 - Abs(dsel - iota))
                        a_t = sb.tile([P, P], F32, tag="oha", bufs=4, name="a_t")
                        nc.scalar.activation(
                            a_t[:], iota_sb[:],
                            mybir.ActivationFunctionType.Abs,
                            bias=dsel_sb[:, j : j + 1], scale=-1.0,
                        )
                        nc.scalar.activation(
                            s_t[:], a_t[:],
                            mybir.ActivationFunctionType.Relu,
                            bias=1.0, scale=-1.0,
                        )
                    else:
                        nc.vector.tensor_scalar(
                            out=s_t[:], in0=iota_sb[:],
                            scalar1=dsel_sb[:, j : j + 1], scalar2=None,
                            op0=mybir.AluOpType.is_equal,
                        )
                    if b not in psums:
                        psums[b] = scp.tile([P, D], F32, tag=f"sc{b % BB}", name=f"scps{b % BB}")
                    nc.tensor.matmul(
                        out=psums[b][:],
                        lhsT=s_t[:],
                        rhs=g_tiles[k][:, col * D : (col + 1) * D],
                        start=(j == S["first_pair"][b]),
                        stop=(j == S["last_pair"][b]),
                    )
                    if j == S["last_pair"][b]:
                        evac_fn(b, psums.pop(b))

            # ------- layers -------
            psum_r = rdp.tile([P, 512], F32, tag="rd")
            for l in range(4):
                if stopped:
                    break
                D = DIN[l]
                nkc = D // P

                def evac_a(b, ps, l=l):
                    rows = S["block_rows"][b]
                    ev = sb.tile([P, D], F32, tag="ev")
                    nc.any.tensor_scalar(
                        out=ev[:], in0=ps[:], scalar1=ncol(0, b), scalar2=None,
                        op0=mybir.AluOpType.mult,
                    )
                    nc.sync.dma_start(
                        out=y1s[l][b * P : b * P + rows, :], in_=ev[:rows]
                    )

                scatter_unit(y0f[l], D, evac_a)
                ag(y1s[l], y1f[l])
                if KSTAGE == 10 + l:
                    dump_and_stop(y1f[l])
                    stopped = True
                    break

                def evac_b(b, ps, l=l, nkc=nkc, D=D):
                    rows = S["block_rows"][b]
                    tb = sb.tile([P, D], F32, tag="tb")
                    nc.any.tensor_scalar(
                        out=tb[:], in0=ps[:], scalar1=ncol(1, b), scalar2=None,
                        op0=mybir.AluOpType.mult,
                    )
                    y0b = sb.tile([P, D], F32, tag="y0b")
                    y1b = sb.tile([P, D], F32, tag="y1b")
                    if rows < P:
                        nc.any.memset(y0b[:], 0.0)
                        nc.any.memset(y1b[:], 0.0)
                    nc.sync.dma_start(
                        out=y0b[:rows], in_=y0s[l][b * P : b * P + rows, :]
                    )
                    nc.sync.dma_start(
                        out=y1b[:rows], in_=y1s[l][b * P : b * P + rows, :]
                    )
                    y2b = sb.tile([P, D], F32, tag="y2b")
                    nc.any.tensor_tensor(
                        out=y2b[:], in0=tb[:], in1=y0b[:],
                        op=mybir.AluOpType.subtract,
                    )
                    # transposes -> feature-major lhsT chunks
                    yts = []
                    for term, ysrc in enumerate((y0b, y1b, y2b)):
                        for kc in range(nkc):
                            tp = tpp.tile([P, P], F32, tag="tp")
                            nc.tensor.transpose(
                                out=tp[:],
                                in_=ysrc[:, kc * P : (kc + 1) * P],
                                identity=ident_sb[:],
                            )
                            yt = sb.tile([P, P], F32, tag="yt", bufs=8, name="yt")
                            nc.any.tensor_copy(out=yt[:], in_=tp[:])
                            yts.append(yt)
                    ph = pp.tile([P, DOUT[l]], F32, tag="dh")
                    nchk = 3 * nkc
                    for j2 in range(nchk):
                        nc.tensor.matmul(
                            out=ph[:],
                            lhsT=yts[j2][:],
                            rhs=w_sb[l][:, j2 * DOUT[l] : (j2 + 1) * DOUT[l]],
                            start=(j2 == 0),
                            stop=(j2 == nchk - 1),
                        )
                    t1 = sb.tile([P, DOUT[l]], F32, tag="t1")
                    nc.any.tensor_scalar(
                        out=t1[:], in0=ph[:], scalar1=ncol(2, b), scalar2=None,
                        op0=mybir.AluOpType.mult,
                    )
                    t2 = sb.tile([P, DOUT[l]], F32, tag="t2")
                    nc.any.tensor_tensor(
                        out=t2[:], in0=t1[:], in1=bt_sb[l][:],
                        op=mybir.AluOpType.add,
                    )
                    if l < 3:
                        o = sb.tile([P, DOUT[l]], F32, tag="lo")
                        nc.scalar.activation(
                            o[:], t2[:], mybir.ActivationFunctionType.Relu,
                            scale=ncol(3, b),
                        )
                        nc.sync.dma_start(
                            out=y0s[l + 1][b * P : b * P + rows, :], in_=o[:rows]
                        )
                    else:
                        h = sb.tile([P, 512], F32, tag="h")
                        nc.scalar.activation(
                            h[:], t2[:], mybir.ActivationFunctionType.Relu,
                        )
                        gs = sb.tile([P, P], F32, tag="gs")
                        nc.any.tensor_scalar(
                            out=gs[:], in0=iota_sb[:],
                            scalar1=gsel_sb[:, b : b + 1], scalar2=None,
                            op0=mybir.AluOpType.is_equal,
                        )
                        nc.tensor.matmul(
                            out=psum_r[:],
                            lhsT=gs[:],
                            rhs=h[:],
                            start=(b == 0),
                            stop=(b == NB - 1),
                        )

                scatter_unit(y1f[l], D, evac_b)
                if l < 3:
                    ag(y0s[l + 1], y0f[l + 1])
                if KSTAGE == 20 + l:
                    if l < 3:
                        dump_and_stop(y0f[l + 1])
                    stopped = True
                    break

            # ------- readout + MLP -------
            if stopped:
                hgp = None
            else:
                hgp = sb.tile([P, 512], F32, tag="hgp")
            if not stopped:
                nc.any.tensor_copy(out=hgp[:], in_=psum_r[:])
                part_d = dram.tile([P, 512], F32, tag="part")
                tot_d = dram.tile([P, 512], F32, tag="tot")
                nc.sync.dma_start(out=part_d[:], in_=hgp[:])
                if KSINGLE:
                    nc.sync.dma_start(out=tot_d[:], in_=part_d[:])
                else:
                    nc.gpsimd.collective_compute(
                        "AllReduce", mybir.AluOpType.add,
                        replica_groups=RG, ins=[part_d[:]], outs=[tot_d[:]],
                    )
                hg = sb.tile([P, 512], F32, tag="hg")
                nc.sync.dma_start(out=hg[:], in_=tot_d[:])

            def mm_block(x_sb, wtile, dout, bias_tile, relu):
                # out = act(x @ W + b): x [128, 512] -> 4 transposed chunks
                xts = []
                for kc in range(4):
                    tp = tpp.tile([P, P], F32, tag="tp")
                    nc.tensor.transpose(
                        out=tp[:], in_=x_sb[:, kc * P : (kc + 1) * P],
                        identity=ident_sb[:],
                    )
                    xt = sb.tile([P, P], F32, tag="yt", bufs=8, name="xt")
                    nc.any.tensor_copy(out=xt[:], in_=tp[:])
                    xts.append(xt)
                ph = pp.tile([P, dout], F32, tag="dh")
                for kc in range(4):
                    nc.tensor.matmul(
                        out=ph[:], lhsT=xts[kc][:],
                        rhs=wtile[:, kc * dout : (kc + 1) * dout],
                        start=(kc == 0), stop=(kc == 3),
                    )
                o = sb.tile([P, dout], F32, tag=f"mo{dout}")
                nc.any.tensor_tensor(
                    out=o[:], in0=ph[:], in1=bias_tile[:], op=mybir.AluOpType.add
                )
                if relu:
                    r = sb.tile([P, dout], F32, tag=f"mr{dout}")
                    nc.scalar.activation(
                        r[:], o[:], mybir.ActivationFunctionType.Relu
                    )
                    return r
                return o

            if not stopped:
                m1 = mm_block(hg, wm1_sb, 512, bm1_sb, relu=True)
                m2 = mm_block(m1, wm2_sb, 16, bm2_sb, relu=False)
                nc.sync.dma_start(out=out[:, :], in_=m2[:])

    nc.finalize()
    return nc


_CACHE = {}
G_OVERRIDE = None      # test hook (reference uses G=128)
TRACE = False          # test hook: request NTFF profiling
LAST_RESULTS = None    # test hook: BassKernelResults of the last run


def kernel(signal, W0, b0, W1, b1, W2, b2, W3, b3, Wm1, bm1, Wm2, bm2,
           src, dst, graph_ids):
    global LAST_RESULTS
    signal = np.ascontiguousarray(np.asarray(signal, dtype=np.float32))
    src = np.asarray(src).astype(np.int64)
    dst = np.asarray(dst).astype(np.int64)
    graph_ids = np.asarray(graph_ids).astype(np.int64)
    N = signal.shape[0]
    G = G_OVERRIDE or 128

    key = (N, src.shape[0], G, hash(src.tobytes()) ^ hash(dst.tobytes())
           ^ hash(graph_ids.tobytes()))
    if key in _CACHE:
        S, nc = _CACHE[key]
    else:
        S = _preprocess(src, dst, graph_ids, N, G)
        nc = _build(S)
        _CACHE.clear()
        _CACHE[key] = (S, nc)

    in_maps = _make_inmaps(
        S, signal, W0, b0, W1, b1, W2, b2, W3, b3, Wm1, bm1, Wm2, bm2
    )

    res = run_bass_kernel_spmd(
        nc, in_maps, core_ids=list(range(NCORES)), trace=TRACE
    )
    LAST_RESULTS = res
    return np.asarray(res.results[0]["out"][:G, :10])


def _make_inmaps(S, signal, W0, b0, W1, b1, W2, b2, W3, b3, Wm1, bm1, Wm2, bm2):
    N = signal.shape[0]
    NLOC = S["NLOC"]
    iota_np = np.broadcast_to(
        np.arange(P, dtype=np.float32)[None, :], (P, P)
    ).copy()
    ident_np = np.eye(P, dtype=np.float32)
    ws = [np.asarray(w, dtype=np.float32) for w in (W0, W1, W2, W3)]
    bts = [
        np.broadcast_to(np.asarray(b, dtype=np.float32)[None, :], (P, len(b))).copy()
        for b in (b0, b1, b2, b3)
    ]
    wm2_p = np.zeros((512, 16), np.float32)
    wm2_p[:, :10] = np.asarray(Wm2, dtype=np.float32)
    bm2_p = np.zeros((P, 16), np.float32)
    bm2_p[:, :10] = np.asarray(bm2, dtype=np.float32)[None, :]
    bm1_t = np.broadcast_to(
        np.asarray(bm1, dtype=np.float32)[None, :], (P, 512)
    ).copy()

    in_maps = []
    for c in range(NCORES):
        lo = c * NLOC
        hi = min(N, lo + NLOC)
        shard = np.zeros((NLOC, 128), np.float32)
        shard[: hi - lo] = signal[lo:hi]
        m = {
            "sig": shard,
            "idx16": S["idx16"][c],
            "dstsel": S["dstsel"][c],
            "normc": S["normc"][c],
            "gsel": S["gsel"][c],
            "iota": iota_np,
            "ident": ident_np,
            "Wm1": np.asarray(Wm1, dtype=np.float32),
            "Bm1": bm1_t,
            "Wm2": wm2_p,
            "Bm2": bm2_p,
        }
        for l in range(4):
            m[f"W{l}"] = ws[l]
            m[f"Bt{l}"] = bts[l]
        in_maps.append(m)
    return in_maps



# revision 15
# speedup vs baseline: 1.1512x; 1.1512x over previous
"""Trainium2 Bass kernel for the DGL-style ChebConv GNN classifier (v2).

Strategy (8 NeuronCores, SPMD):
  - Nodes sharded contiguously (12.5K/core), split into half-shards A (6272
    rows = 49 blocks) and B (6228 rows) so table AllGathers pipeline with
    compute at half-shard granularity.
  - All tables, gathers and matmuls in bf16 (PSUM accumulation fp32).
  - Per sparse pass: dma_gather (bf16 rows, 1024-idx calls, 4 SWDGE queues
    round-robin with a deep tile pool so the 4 Q7 pairs generate descriptors
    concurrently), then segment-sum via one-hot matmuls into per-block PSUM.
  - One-hot matrices are built once on DVE (is_equal vs iota) and cached in
    DRAM; each pass streams them back via HWDGE, freeing DVE.
  - Dense (concat @ W) matmuls consume PE-transposed bf16 blocks.
  - Readout via per-core graph-id one-hot matmul partial sums + fp32
    AllReduce, then the small MLP classifier on-chip.

kernel(**inputs) takes FULL unsharded inputs, returns FULL [G, 10] output.
"""

import os

import numpy as np

import concourse.bacc as bacc
import concourse.mybir as mybir
import concourse.tile as tile
from concourse.bass_utils import run_bass_kernel_spmd

NCORES = 8
P = 128
NLOC = 12500
NB = 98               # blocks per core
HA_BLK = 49
HA = HA_BLK * P       # 6272 rows, half A
HB = NLOC - HA        # 6228 rows, half B
CHUNK = 32768
MAX_CALL = 1024
F32 = mybir.dt.float32
BF16 = mybir.dt.bfloat16
I16 = mybir.dt.int16

DIN = [128, 128, 128, 256]
DOUT = [128, 128, 256, 512]
BB128 = int(os.environ.get("KBB128", "3"))
BB256 = int(os.environ.get("KBB256", "3"))


def _wrap16(local_idx):
    L = local_idx.shape[0]
    w = local_idx.reshape(L // 16, 16).T.copy()
    return np.tile(w, (8, 1))


def _node_map(N):
    """node id -> (window 0..3, idx within window) over the reindexed tables.

    tabA rows: core c half-A local i (<HA) at c*HA+i; windows [0,32768),
    [32768,50176). tabB rows: c*HB + (i-HA); windows [0,32768), [32768,49824).
    """
    v = np.arange(N, dtype=np.int64)
    c = v // NLOC
    i = v % NLOC
    in_a = i < HA
    trow = np.where(in_a, c * HA + i, c * HB + (i - HA))
    hi = trow >= CHUNK
    win = np.where(in_a, 0, 2) + hi
    widx = trow - hi * CHUNK
    return win, widx


def _edge_struct(src, dst, BB):
    """Per-width-class slot/call/pair structure (shared across cores)."""
    win, widx = _node_map(100000)
    NBATCH = (NB + BB - 1) // BB
    NW = 4
    per_core = []
    counts = np.zeros((NCORES, NBATCH, NW), dtype=np.int64)
    core_of = dst // NLOC
    for c in range(NCORES):
        m = core_of == c
        s = src[m]
        dl = dst[m] - c * NLOC
        bat = (dl // P) // BB
        w = win[s]
        order = np.lexsort((dl, w, bat))
        s, dl, bat, w = s[order], dl[order], bat[order], w[order]
        key = bat * NW + w
        counts[c] = np.bincount(key, minlength=NBATCH * NW).reshape(NBATCH, NW)
        per_core.append((s, dl, key))

    runlen = ((counts.max(axis=0) + P - 1) // P) * P     # [NBATCH, NW]
    run_off = np.zeros((NBATCH, NW), dtype=np.int64)
    tot = 0
    for t in range(NBATCH):
        for w in range(NW):
            run_off[t, w] = tot
            tot += runlen[t, w]
    TOT = int(tot)
    TOT = ((TOT + 1023) // 1024) * 1024   # keep idx buffer 16-col aligned
    NSUB = TOT // P

    slot_widx = np.zeros((NCORES, TOT), dtype=np.int64)
    slot_dstl = np.full((NCORES, TOT), -1, dtype=np.int64)
    slot_win = np.zeros(TOT, dtype=np.int64)
    for t in range(NBATCH):
        for w in range(NW):
            o, L = run_off[t, w], runlen[t, w]
            slot_win[o : o + L] = w
    for c in range(NCORES):
        s, dl, key = per_core[c]
        kcnt = counts[c].reshape(-1)
        koff = run_off.reshape(-1)
        pos = np.empty(len(key), dtype=np.int64)
        start = 0
        for k in range(NBATCH * NW):
            n = kcnt[k]
            pos[start : start + n] = koff[k] + np.arange(n)
            start += n
        slot_widx[c][pos] = widx[s]
        slot_dstl[c][pos] = dl

    calls = []      # (window, slot_off, len)
    for t in range(NBATCH):
        for w in range(NW):
            o, L = int(run_off[t, w]), int(runlen[t, w])
            while L > 0:
                seg = min(L, MAX_CALL)
                calls.append((w, o, seg))
                o += seg
                L -= seg

    idx16 = np.zeros((NCORES, P, TOT // 16), dtype=np.int16)
    for c in range(NCORES):
        for w, o, L in calls:
            local = slot_widx[c, o : o + L].astype(np.int16)
            idx16[c][:, o // 16 : (o + L) // 16] = _wrap16(local)

    sub_call = np.zeros(NSUB, dtype=np.int64)
    sub_col = np.zeros(NSUB, dtype=np.int64)
    for k, (w, o, L) in enumerate(calls):
        for j in range(L // P):
            sub_call[o // P + j] = k
            sub_col[o // P + j] = j

    blk_all = np.where(slot_dstl >= 0, slot_dstl // P, -1)
    pairs = []
    for sidx in range(NSUB):
        sl = blk_all[:, sidx * P : (sidx + 1) * P]
        present = np.unique(sl[sl >= 0])
        for b in present:
            pairs.append((sidx, int(b)))
    NPAIRS = len(pairs)

    first_pair = {}
    last_pair = {}
    for j, (sidx, b) in enumerate(pairs):
        if b not in first_pair:
            first_pair[b] = j
        last_pair[b] = j

    dsel = np.full((NCORES, P, NPAIRS), -1.0, dtype=np.float32)
    for j, (sidx, b) in enumerate(pairs):
        sl = slot_dstl[:, sidx * P : (sidx + 1) * P]
        m = (sl // P) == b
        dsel[:, :, j] = np.where(m, (sl - b * P).astype(np.float32), -1.0)

    return dict(
        BB=BB, NBATCH=NBATCH, TOT=TOT, NSUB=NSUB, calls=calls,
        pairs=pairs, first_pair=first_pair, last_pair=last_pair,
        sub_call=sub_call, sub_col=sub_col, idx16=idx16, dsel=dsel,
    )


def _preprocess(src, dst, graph_ids, N, G):
    assert N == NCORES * NLOC
    deg = np.bincount(dst, minlength=N).astype(np.float32)
    norm = np.clip(deg, 1.0, None) ** -0.5
    norm2 = norm * norm
    inv_norm = 1.0 / norm

    S128 = _edge_struct(src, dst, BB128)
    S256 = _edge_struct(src, dst, BB256)

    normc = np.zeros((NCORES, P, 4 * NB), dtype=np.float32)
    gsel = np.full((NCORES, P, NB), -1.0, dtype=np.float32)
    for c in range(NCORES):
        lo = c * NLOC
        hi = lo + NLOC
        pad = NB * P - NLOC
        nn = np.pad(norm[lo:hi], (0, pad)).reshape(NB, P).T
        n2 = np.pad(norm2[lo:hi], (0, pad)).reshape(NB, P).T
        iv = np.pad(inv_norm[lo:hi], (0, pad)).reshape(NB, P).T
        normc[c][:, 0 * NB : 1 * NB] = -n2
        normc[c][:, 1 * NB : 2 * NB] = -2.0 * n2
        normc[c][:, 2 * NB : 3 * NB] = iv
        normc[c][:, 3 * NB : 4 * NB] = nn
        gs = np.pad(graph_ids[lo:hi].astype(np.float32), (0, pad),
                    constant_values=-1.0)
        gsel[c] = gs.reshape(NB, P).T

    block_rows = [P] * (NB - 1) + [HB - 48 * P]
    return dict(S128=S128, S256=S256, normc=normc, gsel=gsel,
                block_rows=block_rows, norm=norm)


def _build(S):
    KSTAGE = int(os.environ.get("KSTAGE", "99"))
    S1, S2 = S["S128"], S["S256"]
    NP1, NP2 = len(S1["pairs"]), len(S2["pairs"])
    TOTMX = max(S1["TOT"], S2["TOT"])
    NTA, NTB = NCORES * HA, NCORES * HB

    nc = bacc.Bacc(trn_type="TRN2", num_devices=NCORES,
                   dynamic_dma_scratch_size=32768, num_swdge_queues=4)

    sigA_in = nc.dram_tensor("sigA", [HA, 128], BF16, kind="ExternalInput")
    sigB_in = nc.dram_tensor("sigB", [HB, 128], BF16, kind="ExternalInput")
    idx1_in = nc.dram_tensor("idx128", [P, S1["TOT"] // 16], I16,
                             kind="ExternalInput")
    idx2_in = nc.dram_tensor("idx256", [P, S2["TOT"] // 16], I16,
                             kind="ExternalInput")
    ds1_in = nc.dram_tensor("dsel128", [P, NP1], F32, kind="ExternalInput")
    ds2_in = nc.dram_tensor("dsel256", [P, NP2], F32, kind="ExternalInput")
    normc_in = nc.dram_tensor("normc", [P, 4 * NB], F32, kind="ExternalInput")
    gsel_in = nc.dram_tensor("gsel", [P, NB], F32, kind="ExternalInput")
    iota16_in = nc.dram_tensor("iota16", [P, P], BF16, kind="ExternalInput")
    iotaf_in = nc.dram_tensor("iotaf", [P, P], F32, kind="ExternalInput")
    id16_in = nc.dram_tensor("id16", [P, P], BF16, kind="ExternalInput")
    idf_in = nc.dram_tensor("idf", [P, P], F32, kind="ExternalInput")
    w_in = [nc.dram_tensor(f"W{l}", [3 * DIN[l], DOUT[l]], BF16,
                           kind="ExternalInput") for l in range(4)]
    bt_in = [nc.dram_tensor(f"Bt{l}", [P, DOUT[l]], F32,
                            kind="ExternalInput") for l in range(4)]
    wm1_in = nc.dram_tensor("Wm1", [512, 512], BF16, kind="ExternalInput")
    bm1_in = nc.dram_tensor("Bm1", [P, 512], F32, kind="ExternalInput")
    wm2_in = nc.dram_tensor("Wm2", [512, 16], BF16, kind="ExternalInput")
    bm2_in = nc.dram_tensor("Bm2", [P, 16], F32, kind="ExternalInput")
    out = nc.dram_tensor("out", [P, 16], F32, kind="ExternalOutput")

    RG = [list(range(NCORES))]

    with tile.TileContext(nc) as tc:
        with (
            tc.tile_pool(name="dram", bufs=1, space="DRAM") as dram,
            tc.tile_pool(name="res", bufs=1) as res,
            tc.tile_pool(name="sb", bufs=2) as sb,
            tc.tile_pool(name="scp", bufs=1, space="PSUM") as scp,
            tc.tile_pool(name="pp", bufs=1, space="PSUM") as pp,
            tc.tile_pool(name="tpp", bufs=1, space="PSUM") as tpp,
            tc.tile_pool(name="rdp", bufs=1, space="PSUM") as rdp,
        ):
            # ---------- resident ----------
            idx_sb = res.tile([P, TOTMX // 16], I16, tag="idx")
            nc.sync.dma_start(out=idx_sb[:, : S1["TOT"] // 16],
                              in_=idx1_in[:, :])
            normc_sb = res.tile([P, 4 * NB], F32)
            gsel_sb = res.tile([P, NB], F32)
            iota16_sb = res.tile([P, P], BF16)
            iotaf_sb = res.tile([P, P], F32)
            id16_sb = res.tile([P, P], BF16)
            idf_sb = res.tile([P, P], F32)
            for t, src_t in ((normc_sb, normc_in), (gsel_sb, gsel_in),
                             (iota16_sb, iota16_in), (iotaf_sb, iotaf_in),
                             (id16_sb, id16_in), (idf_sb, idf_in)):
                nc.sync.dma_start(out=t[:], in_=src_t[:, :])
            w_sb = []
            for l in range(4):
                nchk = 3 * DIN[l] // P
                t = res.tile([P, nchk * DOUT[l]], BF16, tag=f"W{l}")
                for j in range(nchk):
                    nc.sync.dma_start(
                        out=t[:, j * DOUT[l] : (j + 1) * DOUT[l]],
                        in_=w_in[l][j * P : (j + 1) * P, :])
                w_sb.append(t)
            bt_sb = []
            for l in range(4):
                t = res.tile([P, DOUT[l]], F32, tag=f"Bt{l}")
                nc.sync.dma_start(out=t[:], in_=bt_in[l][:, :])
                bt_sb.append(t)
            wm1_sb = res.tile([P, 4 * 512], BF16)
            for j in range(4):
                nc.sync.dma_start(out=wm1_sb[:, j * 512 : (j + 1) * 512],
                                  in_=wm1_in[j * P : (j + 1) * P, :])
            bm1_sb = res.tile([P, 512], F32)
            nc.sync.dma_start(out=bm1_sb[:], in_=bm1_in[:, :])
            wm2_sb = res.tile([P, 4 * 16], BF16)
            for j in range(4):
                nc.sync.dma_start(out=wm2_sb[:, j * 16 : (j + 1) * 16],
                                  in_=wm2_in[j * P : (j + 1) * P, :])
            bm2_sb = res.tile([P, 16], F32)
            nc.sync.dma_start(out=bm2_sb[:], in_=bm2_in[:, :])

            # ---------- DRAM tiles ----------
            def dt(name, rows, d):
                return dram.tile([rows, d], BF16, tag=name, name=name,
                                 addr_space="Shared")

            # Shared tiles are single-writer: one table pair per (kind, layer)
            tabs = {}
            for l in range(3):
                tabs[("y0", 128, l)] = (dt(f"tA0_128_{l}", NTA, 128),
                                        dt(f"tB0_128_{l}", NTB, 128))
                tabs[("y1", 128, l)] = (dt(f"tA1_128_{l}", NTA, 128),
                                        dt(f"tB1_128_{l}", NTB, 128))
            tabs[("y0", 256, 3)] = (dt("tA0_256", NTA, 256),
                                    dt("tB0_256", NTB, 256))
            tabs[("y1", 256, 3)] = (dt("tA1_256", NTA, 256),
                                    dt("tB1_256", NTB, 256))
            # y0_128 shards are ping-ponged between layers so layer l+1's
            # writes don't serialize against layer l's block reads.
            shards = {
                ("y0", 128, 0): (dram.tile([HA, 128], BF16, name="shA0_128p0"),
                                 dram.tile([HB, 128], BF16, name="shB0_128p0")),
                ("y0", 128, 1): (dram.tile([HA, 128], BF16, name="shA0_128p1"),
                                 dram.tile([HB, 128], BF16, name="shB0_128p1")),
                ("y1", 128): (dram.tile([HA, 128], BF16, name="shA1_128"),
                              dram.tile([HB, 128], BF16, name="shB1_128")),
                ("y0", 256): (dram.tile([HA, 256], BF16, name="shA0_256"),
                              dram.tile([HB, 256], BF16, name="shB0_256")),
                ("y1", 256): (dram.tile([HA, 256], BF16, name="shA1_256"),
                              dram.tile([HB, 256], BF16, name="shB1_256")),
            }
            oh1_d = dram.tile([P, NP1 * P], BF16, name="oh128")
            oh2_d = dram.tile([P, NP2 * P], BF16, name="oh256")

            def ag_pair(shkey, tabkey):
                shA, shB = shards[shkey]
                tA, tB = tabs[tabkey]
                for sh, t in ((shA, tA), (shB, tB)):
                    nc.gpsimd.collective_compute(
                        "AllGather", mybir.AluOpType.bypass,
                        replica_groups=RG, ins=[sh[:]], outs=[t[:]])

            def ncol(kind, b):
                return normc_sb[:, kind * NB + b : kind * NB + b + 1]

            # ---------- one-hot cache build ----------
            ds_sb = res.tile([P, max(NP1, NP2)], F32, tag="dsel")
            nc.sync.dma_start(out=ds_sb[:, :NP1], in_=ds1_in[:, :])
            with nc.named_scope("ohbuild"):
                for grp in range(0, NP1, 16):
                    n = min(16, NP1 - grp)
                    s = sb.tile([P, 16 * P], BF16, tag="ohb", bufs=3)
                    for jj in range(n):
                        nc.vector.tensor_scalar(
                            out=s[:, jj * P : (jj + 1) * P], in0=iota16_sb[:],
                            scalar1=ds_sb[:, grp + jj : grp + jj + 1],
                            scalar2=None, op0=mybir.AluOpType.is_equal)
                    nc.sync.dma_start(
                        out=oh1_d[:, grp * P : (grp + n) * P],
                        in_=s[:, : n * P])
                # class 256 dsel overwrites the same SBUF tile
                nc.sync.dma_start(out=ds_sb[:, :NP2], in_=ds2_in[:, :])
                for grp in range(0, NP2, 16):
                    n = min(16, NP2 - grp)
                    s = sb.tile([P, 16 * P], BF16, tag="ohb", bufs=3)
                    for jj in range(n):
                        nc.vector.tensor_scalar(
                            out=s[:, jj * P : (jj + 1) * P], in0=iota16_sb[:],
                            scalar1=ds_sb[:, grp + jj : grp + jj + 1],
                            scalar2=None, op0=mybir.AluOpType.is_equal)
                    nc.sync.dma_start(
                        out=oh2_d[:, grp * P : (grp + n) * P],
                        in_=s[:, : n * P])

            # ---------- signal -> y0 tables ----------
            shA0, shB0 = shards[("y0", 128, 0)]
            nc.sync.dma_start(out=shA0[:, :], in_=sigA_in[:, :])
            nc.sync.dma_start(out=shB0[:, :], in_=sigB_in[:, :])
            ag_pair(("y0", 128, 0), ("y0", 128, 0))

            stopped = [KSTAGE == 0]

            def dump_and_stop(tab):
                d = sb.tile([P, 16], F32, tag="dmp", name="dmp")
                g = sb.tile([P, 16], BF16, tag="dmpg", name="dmpg")
                nc.sync.dma_start(out=g[:], in_=tab[0:P, 0:16])
                nc.vector.tensor_copy(out=d[:], in_=g[:])
                nc.sync.dma_start(out=out[:, :], in_=d[:])

            if stopped[0]:
                dump_and_stop(tabs[("y0", 128, 0)][0])

            # ---------- scatter unit ----------
            def scatter_unit(SS, tabpair, D, oh_d, evac_fn, uname):
                tA, tB = tabpair
                windows = [(tA, 0, CHUNK), (tA, CHUNK, NTA - CHUNK),
                           (tB, 0, CHUNK), (tB, CHUNK, NTB - CHUNK)]
                g_tiles = {}
                psums.clear()
                emitted = -1
                with nc.named_scope(uname):
                    for j, (sidx, b) in enumerate(SS["pairs"]):
                        k = int(SS["sub_call"][sidx])
                        if k > emitted:
                            for kk in range(emitted + 1, k + 1):
                                w, o, L = SS["calls"][kk]
                                tab, tlo, trows = windows[w]
                                g = sb.tile([P, (MAX_CALL // P) * 256], BF16,
                                            tag="g", bufs=10, name="g")
                                nc.gpsimd.dma_gather(
                                    out_ap=g[:, : (L // P) * D].rearrange(
                                        "p (k d) -> p k d", d=D),
                                    in_ap=tab[tlo : tlo + trows, :],
                                    idxs_ap=idx_sb[:, o // 16 : (o + L) // 16],
                                    num_idxs=L,
                                    num_idxs_reg=L,
                                    elem_size=D,
                                    queue_num=kk % int(os.environ.get("KQ", "4")),
                                )
                                g_tiles[kk] = g
                            emitted = k
                        col = int(SS["sub_col"][sidx])
                        oh = sb.tile([P, P], BF16, tag="oh", bufs=24, name="oh")
                        nc.sync.dma_start(
                            out=oh[:], in_=oh_d[:, j * P : (j + 1) * P])
                        bk = b % SS["BB"]
                        acc_pb = 1  # one accumulation group per PSUM bank
                        bank = bk // acc_pb
                        cs = (bk % acc_pb) * D
                        bkey = (b // SS["BB"], bank)
                        if bkey not in psums:
                            psums[bkey] = scp.tile([P, 512], F32,
                                                   tag=f"scb{bank}",
                                                   name=f"scb{bank}")
                        nc.tensor.matmul(
                            out=psums[bkey][:, cs : cs + D],
                            lhsT=oh[:],
                            rhs=g_tiles[k][:, col * D : (col + 1) * D],
                            start=(j == SS["first_pair"][b]),
                            stop=(j == SS["last_pair"][b]),
                        )
                        if j == SS["last_pair"][b]:
                            evac_fn(b, psums[bkey][:, cs : cs + D])

            psums = {}

            def sh_dst(b):
                if b < HA_BLK:
                    return 0, b * P
                return 1, (b - HA_BLK) * P

            psum_r = rdp.tile([P, 512], F32, tag="rd")

            def evac_a(b, ps, width, l):
                rows = S["block_rows"][b]
                D = width
                half, off = sh_dst(b)
                sh = shards[("y1", width)][half]
                ev = sb.tile([P, 256], BF16, tag="ev", bufs=4)
                nc.any.tensor_scalar(
                    out=ev[:, :D], in0=ps, scalar1=ncol(0, b),
                    scalar2=None, op0=mybir.AluOpType.mult)
                nc.sync.dma_start(out=sh[off : off + rows, :],
                                  in_=ev[:rows, :D])
                if b == HA_BLK - 1:
                    nc.gpsimd.collective_compute(
                        "AllGather", mybir.AluOpType.bypass, replica_groups=RG,
                        ins=[shards[("y1", width)][0][:]],
                        outs=[tabs[("y1", width, l)][0][:]])
                if b == NB - 1:
                    nc.gpsimd.collective_compute(
                        "AllGather", mybir.AluOpType.bypass, replica_groups=RG,
                        ins=[shards[("y1", width)][1][:]],
                        outs=[tabs[("y1", width, l)][1][:]])

            def evac_b(b, ps, l):
                D = DIN[l]
                Do = DOUT[l]
                nkc = D // P
                rows = S["block_rows"][b]
                half, off = sh_dst(b)
                tb = sb.tile([P, 256], BF16, tag="tb", bufs=2)
                nc.any.tensor_scalar(
                    out=tb[:, :D], in0=ps, scalar1=ncol(1, b),
                    scalar2=None, op0=mybir.AluOpType.mult)
                y0b = sb.tile([P, 256], BF16, tag="y0b", bufs=2)
                y1b = sb.tile([P, 256], BF16, tag="y1b", bufs=2)
                if rows < P:
                    nc.any.memset(y0b[:], 0.0)
                    nc.any.memset(y1b[:], 0.0)
                y0key = ("y0", 256) if D == 256 else ("y0", 128, l % 2)
                nc.sync.dma_start(
                    out=y0b[:rows, :D],
                    in_=shards[y0key][half][off : off + rows, :])
                nc.sync.dma_start(
                    out=y1b[:rows, :D],
                    in_=shards[("y1", D)][half][off : off + rows, :])
                y2b = sb.tile([P, 256], BF16, tag="y2b", bufs=2)
                nc.any.tensor_tensor(
                    out=y2b[:, :D], in0=tb[:, :D], in1=y0b[:, :D],
                    op=mybir.AluOpType.subtract)
                yts = []
                for term, ysrc in enumerate((y0b, y1b, y2b)):
                    for kc in range(nkc):
                        tp = tpp.tile([P, P], BF16, tag="tp16", bufs=2)
                        nc.tensor.transpose(
                            out=tp[:], in_=ysrc[:, kc * P : (kc + 1) * P],
                            identity=id16_sb[:])
                        yt = sb.tile([P, P], BF16, tag="yt", bufs=8, name="yt")
                        if (term * nkc + kc) % 2 == 0:
                            nc.vector.tensor_copy(out=yt[:], in_=tp[:])
                        else:
                            nc.scalar.copy(out=yt[:], in_=tp[:])
                        yts.append(yt)
                ph = pp.tile([P, 512], F32, tag="dh")
                nchk = 3 * nkc
                for j2 in range(nchk):
                    nc.tensor.matmul(
                        out=ph[:, :Do], lhsT=yts[j2][:],
                        rhs=w_sb[l][:, j2 * Do : (j2 + 1) * Do],
                        start=(j2 == 0), stop=(j2 == nchk - 1))
                t1 = sb.tile([P, 512], F32, tag="t1", bufs=2)
                nc.any.tensor_scalar(
                    out=t1[:, :Do], in0=ph[:, :Do], scalar1=ncol(2, b),
                    scalar2=None, op0=mybir.AluOpType.mult)
                t2 = sb.tile([P, 512], F32, tag="t2", bufs=2)
                nc.any.tensor_tensor(
                    out=t2[:, :Do], in0=t1[:, :Do], in1=bt_sb[l][:],
                    op=mybir.AluOpType.add)
                if l < 3:
                    wo = DOUT[l]
                    o_t = sb.tile([P, 256], BF16, tag="lo", bufs=2)
                    nc.scalar.activation(
                        o_t[:, :wo], t2[:, :wo],
                        mybir.ActivationFunctionType.Relu,
                        scale=ncol(3, b))
                    wkey = (("y0", 256) if wo >= 256
                            else ("y0", 128, (l + 1) % 2))
                    wclass = 256 if wo >= 256 else 128
                    sh = shards[wkey][half]
                    nc.sync.dma_start(out=sh[off : off + rows, :],
                                      in_=o_t[:rows, :wo])
                    if b == HA_BLK - 1:
                        nc.gpsimd.collective_compute(
                            "AllGather", mybir.AluOpType.bypass,
                            replica_groups=RG,
                            ins=[shards[wkey][0][:]],
                            outs=[tabs[("y0", wclass, l + 1)][0][:]])
                    if b == NB - 1:
                        nc.gpsimd.collective_compute(
                            "AllGather", mybir.AluOpType.bypass,
                            replica_groups=RG,
                            ins=[shards[wkey][1][:]],
                            outs=[tabs[("y0", wclass, l + 1)][1][:]])
                else:
                    h = sb.tile([P, 512], BF16, tag="h", bufs=2)
                    nc.scalar.activation(
                        h[:], t2[:], mybir.ActivationFunctionType.Relu)
                    gs = sb.tile([P, P], BF16, tag="gs", bufs=2)
                    nc.vector.tensor_scalar(
                        out=gs[:], in0=iota16_sb[:],
                        scalar1=gsel_sb[:, b : b + 1], scalar2=None,
                        op0=mybir.AluOpType.is_equal)
                    nc.tensor.matmul(
                        out=psum_r[:], lhsT=gs[:], rhs=h[:],
                        start=(b == 0), stop=(b == NB - 1))

            # ---------- layers ----------
            for l in range(4):
                if stopped[0]:
                    break
                width = DIN[l]
                SS = S["S128"] if width == 128 else S["S256"]
                oh_d = oh1_d if width == 128 else oh2_d
                if l == 3:
                    # swap in the 256-class indices
                    nc.sync.dma_start(out=idx_sb[:, : S2["TOT"] // 16],
                                      in_=idx2_in[:, :])
                scatter_unit(SS, tabs[("y0", width, l)], width, oh_d,
                             lambda b, ps, wd=width, ll=l:
                                 evac_a(b, ps, wd, ll),
                             f"u{l}A")
                if KSTAGE == 10 + l:
                    dump_and_stop(tabs[("y1", width, l)][0])
                    stopped[0] = True
                    break
                scatter_unit(SS, tabs[("y1", width, l)], width, oh_d,
                             lambda b, ps, ll=l: evac_b(b, ps, ll),
                             f"u{l}B")
                if KSTAGE == 20 + l:
                    if l < 3:
                        dump_and_stop(
                            tabs[("y0", 256 if DOUT[l] >= 256 else 128,
                                  l + 1)][0])
                    stopped[0] = True
                    break

            # ---------- readout + MLP ----------
            if not stopped[0]:
                hgp = sb.tile([P, 512], F32, tag="hgp")
                nc.any.tensor_copy(out=hgp[:], in_=psum_r[:])
                part_d = dram.tile([P, 512], F32, tag="part", name="part")
                tot_d = dram.tile([P, 512], F32, tag="tot", name="tot",
                                  addr_space="Shared")
                nc.sync.dma_start(out=part_d[:], in_=hgp[:])
                nc.gpsimd.collective_compute(
                    "AllReduce", mybir.AluOpType.add,
                    replica_groups=RG, ins=[part_d[:]], outs=[tot_d[:]])
                hg = sb.tile([P, 512], F32, tag="hg")
                nc.sync.dma_start(out=hg[:], in_=tot_d[:])

                def mm_block(x_sb, wtile, dout, bias_tile, relu):
                    xts = []
                    for kc in range(4):
                        tp = tpp.tile([P, P], F32, tag="tpf", bufs=1)
                        nc.tensor.transpose(
                            out=tp[:], in_=x_sb[:, kc * P : (kc + 1) * P],
                            identity=idf_sb[:])
                        xt = sb.tile([P, P], BF16, tag="yt", bufs=8, name="xt")
                        nc.any.tensor_copy(out=xt[:], in_=tp[:])
                        xts.append(xt)
                    ph = pp.tile([P, 512], F32, tag="dh")
                    for kc in range(4):
                        nc.tensor.matmul(
                            out=ph[:, :dout], lhsT=xts[kc][:],
                            rhs=wtile[:, kc * dout : (kc + 1) * dout],
                            start=(kc == 0), stop=(kc == 3))
                    o_t = sb.tile([P, dout], F32, tag=f"mo{dout}")
                    nc.any.tensor_tensor(
                        out=o_t[:], in0=ph[:, :dout], in1=bias_tile[:],
                        op=mybir.AluOpType.add)
                    if relu:
                        r = sb.tile([P, dout], F32, tag=f"mr{dout}")
                        nc.scalar.activation(
                            r[:], o_t[:], mybir.ActivationFunctionType.Relu)
                        return r
                    return o_t

                m1 = mm_block(hg, wm1_sb, 512, bm1_sb, relu=True)
                m2 = mm_block(m1, wm2_sb, 16, bm2_sb, relu=False)
                nc.sync.dma_start(out=out[:, :], in_=m2[:])

    nc.finalize()
    return nc


_CACHE = {}
G_OVERRIDE = None
TRACE = False
LAST_RESULTS = None


def kernel(signal, W0, b0, W1, b1, W2, b2, W3, b3, Wm1, bm1, Wm2, bm2,
           src, dst, graph_ids):
    global LAST_RESULTS
    import ml_dtypes

    signal = np.ascontiguousarray(np.asarray(signal, dtype=np.float32))
    src = np.asarray(src).astype(np.int64)
    dst = np.asarray(dst).astype(np.int64)
    graph_ids = np.asarray(graph_ids).astype(np.int64)
    N = signal.shape[0]
    G = G_OVERRIDE or 128

    key = (N, src.shape[0], G, hash(src.tobytes()) ^ hash(dst.tobytes())
           ^ hash(graph_ids.tobytes()))
    if key in _CACHE:
        S, nc = _CACHE[key]
    else:
        S = _preprocess(src, dst, graph_ids, N, G)
        nc = _build(S)
        _CACHE.clear()
        _CACHE[key] = (S, nc)

    S1, S2 = S["S128"], S["S256"]
    y0 = signal * S["norm"][:, None]
    iota_np = np.broadcast_to(np.arange(P, dtype=np.float32)[None, :],
                              (P, P)).copy()
    ident_np = np.eye(P, dtype=np.float32)
    ws = [np.asarray(w, dtype=np.float32).astype(ml_dtypes.bfloat16)
          for w in (W0, W1, W2, W3)]
    bts = [np.broadcast_to(np.asarray(b, np.float32)[None, :],
                           (P, len(b))).copy() for b in (b0, b1, b2, b3)]
    wm2_p = np.zeros((512, 16), np.float32)
    wm2_p[:, :10] = np.asarray(Wm2, dtype=np.float32)
    bm2_p = np.zeros((P, 16), np.float32)
    bm2_p[:, :10] = np.asarray(bm2, dtype=np.float32)[None, :]
    bm1_t = np.broadcast_to(np.asarray(bm1, np.float32)[None, :],
                            (P, 512)).copy()

    in_maps = []
    for c in range(NCORES):
        lo = c * NLOC
        m = {
            "sigA": y0[lo : lo + HA].astype(ml_dtypes.bfloat16),
            "sigB": y0[lo + HA : lo + NLOC].astype(ml_dtypes.bfloat16),
            "idx128": S1["idx16"][c],
            "idx256": S2["idx16"][c],
            "dsel128": S1["dsel"][c],
            "dsel256": S2["dsel"][c],
            "normc": S["normc"][c],
            "gsel": S["gsel"][c],
            "iota16": iota_np.astype(ml_dtypes.bfloat16),
            "iotaf": iota_np,
            "id16": ident_np.astype(ml_dtypes.bfloat16),
            "idf": ident_np,
            "Wm1": np.asarray(Wm1, np.float32).astype(ml_dtypes.bfloat16),
            "Bm1": bm1_t,
            "Wm2": wm2_p.astype(ml_dtypes.bfloat16),
            "Bm2": bm2_p,
        }
        for l in range(4):
            m[f"W{l}"] = ws[l]
            m[f"Bt{l}"] = bts[l]
        in_maps.append(m)

    res = run_bass_kernel_spmd(
        nc, in_maps, core_ids=list(range(NCORES)), trace=TRACE
    )
    LAST_RESULTS = res
    return np.asarray(res.results[0]["out"][:G, :10])


# revision 16
# speedup vs baseline: 1.6704x; 1.4510x over previous
"""Trainium2 Bass kernel for the DGL-style ChebConv GNN classifier (v2).

Strategy (8 NeuronCores, SPMD):
  - Nodes sharded contiguously (12.5K/core), split into half-shards A (6272
    rows = 49 blocks) and B (6228 rows) so table AllGathers pipeline with
    compute at half-shard granularity.
  - All tables, gathers and matmuls in bf16 (PSUM accumulation fp32).
  - Per sparse pass: dma_gather (bf16 rows, 1024-idx calls, 4 SWDGE queues
    round-robin with a deep tile pool so the 4 Q7 pairs generate descriptors
    concurrently), then segment-sum via one-hot matmuls into per-block PSUM.
  - One-hot matrices are built once on DVE (is_equal vs iota) and cached in
    DRAM; each pass streams them back via HWDGE, freeing DVE.
  - Dense (concat @ W) matmuls consume PE-transposed bf16 blocks.
  - Readout via per-core graph-id one-hot matmul partial sums + fp32
    AllReduce, then the small MLP classifier on-chip.

kernel(**inputs) takes FULL unsharded inputs, returns FULL [G, 10] output.
"""

import os

import numpy as np

import concourse.bacc as bacc
import concourse.mybir as mybir
import concourse.tile as tile
from concourse.bass_utils import run_bass_kernel_spmd

NCORES = 8
P = 128
NLOC = 12500
NB = 98               # blocks per core
HA_BLK = 49
HA = HA_BLK * P       # 6272 rows, half A
HB = NLOC - HA        # 6228 rows, half B
CHUNK = 32768
MAX_CALL = 1024
F32 = mybir.dt.float32
BF16 = mybir.dt.bfloat16
I16 = mybir.dt.int16

DIN = [128, 128, 128, 256]
DOUT = [128, 128, 256, 512]
BB128 = int(os.environ.get("KBB128", "3"))
BB256 = int(os.environ.get("KBB256", "3"))


def _wrap16(local_idx):
    L = local_idx.shape[0]
    w = local_idx.reshape(L // 16, 16).T.copy()
    return np.tile(w, (8, 1))


def _node_map(N):
    """node id -> (window 0..3, idx within window) over the reindexed tables.

    tabA rows: core c half-A local i (<HA) at c*HA+i; windows [0,32768),
    [32768,50176). tabB rows: c*HB + (i-HA); windows [0,32768), [32768,49824).
    """
    v = np.arange(N, dtype=np.int64)
    c = v // NLOC
    i = v % NLOC
    in_a = i < HA
    trow = np.where(in_a, c * HA + i, c * HB + (i - HA))
    hi = trow >= CHUNK
    win = np.where(in_a, 0, 2) + hi
    widx = trow - hi * CHUNK
    return win, widx


def _edge_struct(src, dst, BB):
    """Per-width-class slot/call/pair structure (shared across cores)."""
    win, widx = _node_map(100000)
    NBATCH = (NB + BB - 1) // BB
    NW = 4
    per_core = []
    counts = np.zeros((NCORES, NBATCH, NW), dtype=np.int64)
    core_of = dst // NLOC
    for c in range(NCORES):
        m = core_of == c
        s = src[m]
        dl = dst[m] - c * NLOC
        bat = (dl // P) // BB
        w = win[s]
        order = np.lexsort((dl, w, bat))
        s, dl, bat, w = s[order], dl[order], bat[order], w[order]
        key = bat * NW + w
        counts[c] = np.bincount(key, minlength=NBATCH * NW).reshape(NBATCH, NW)
        per_core.append((s, dl, key))

    runlen = ((counts.max(axis=0) + P - 1) // P) * P     # [NBATCH, NW]
    run_off = np.zeros((NBATCH, NW), dtype=np.int64)
    tot = 0
    for t in range(NBATCH):
        for w in range(NW):
            run_off[t, w] = tot
            tot += runlen[t, w]
    TOT = int(tot)
    TOT = ((TOT + 1023) // 1024) * 1024   # keep idx buffer 16-col aligned
    NSUB = TOT // P

    slot_widx = np.zeros((NCORES, TOT), dtype=np.int64)
    slot_dstl = np.full((NCORES, TOT), -1, dtype=np.int64)
    slot_win = np.zeros(TOT, dtype=np.int64)
    for t in range(NBATCH):
        for w in range(NW):
            o, L = run_off[t, w], runlen[t, w]
            slot_win[o : o + L] = w
    for c in range(NCORES):
        s, dl, key = per_core[c]
        kcnt = counts[c].reshape(-1)
        koff = run_off.reshape(-1)
        pos = np.empty(len(key), dtype=np.int64)
        start = 0
        for k in range(NBATCH * NW):
            n = kcnt[k]
            pos[start : start + n] = koff[k] + np.arange(n)
            start += n
        slot_widx[c][pos] = widx[s]
        slot_dstl[c][pos] = dl

    calls = []      # (window, slot_off, len)
    for t in range(NBATCH):
        for w in range(NW):
            o, L = int(run_off[t, w]), int(runlen[t, w])
            while L > 0:
                seg = min(L, MAX_CALL)
                calls.append((w, o, seg))
                o += seg
                L -= seg

    idx16 = np.zeros((NCORES, P, TOT // 16), dtype=np.int16)
    for c in range(NCORES):
        for w, o, L in calls:
            local = slot_widx[c, o : o + L].astype(np.int16)
            idx16[c][:, o // 16 : (o + L) // 16] = _wrap16(local)

    sub_call = np.zeros(NSUB, dtype=np.int64)
    sub_col = np.zeros(NSUB, dtype=np.int64)
    for k, (w, o, L) in enumerate(calls):
        for j in range(L // P):
            sub_call[o // P + j] = k
            sub_col[o // P + j] = j

    blk_all = np.where(slot_dstl >= 0, slot_dstl // P, -1)
    pairs = []
    for sidx in range(NSUB):
        sl = blk_all[:, sidx * P : (sidx + 1) * P]
        present = np.unique(sl[sl >= 0])
        for b in present:
            pairs.append((sidx, int(b)))
    NPAIRS = len(pairs)

    first_pair = {}
    last_pair = {}
    for j, (sidx, b) in enumerate(pairs):
        if b not in first_pair:
            first_pair[b] = j
        last_pair[b] = j

    dsel = np.full((NCORES, P, NPAIRS), -1.0, dtype=np.float32)
    for j, (sidx, b) in enumerate(pairs):
        sl = slot_dstl[:, sidx * P : (sidx + 1) * P]
        m = (sl // P) == b
        dsel[:, :, j] = np.where(m, (sl - b * P).astype(np.float32), -1.0)

    return dict(
        BB=BB, NBATCH=NBATCH, TOT=TOT, NSUB=NSUB, calls=calls,
        pairs=pairs, first_pair=first_pair, last_pair=last_pair,
        sub_call=sub_call, sub_col=sub_col, idx16=idx16, dsel=dsel,
    )


def _preprocess(src, dst, graph_ids, N, G):
    assert N == NCORES * NLOC
    deg = np.bincount(dst, minlength=N).astype(np.float32)
    norm = np.clip(deg, 1.0, None) ** -0.5
    norm2 = norm * norm
    inv_norm = 1.0 / norm

    S128 = _edge_struct(src, dst, BB128)
    S256 = _edge_struct(src, dst, BB256)

    normc = np.zeros((NCORES, P, 4 * NB), dtype=np.float32)
    gsel = np.full((NCORES, P, NB), -1.0, dtype=np.float32)
    for c in range(NCORES):
        lo = c * NLOC
        hi = lo + NLOC
        pad = NB * P - NLOC
        nn = np.pad(norm[lo:hi], (0, pad)).reshape(NB, P).T
        n2 = np.pad(norm2[lo:hi], (0, pad)).reshape(NB, P).T
        iv = np.pad(inv_norm[lo:hi], (0, pad)).reshape(NB, P).T
        normc[c][:, 0 * NB : 1 * NB] = -n2
        normc[c][:, 1 * NB : 2 * NB] = -2.0 * n2
        normc[c][:, 2 * NB : 3 * NB] = iv
        normc[c][:, 3 * NB : 4 * NB] = nn
        gs = np.pad(graph_ids[lo:hi].astype(np.float32), (0, pad),
                    constant_values=-1.0)
        gsel[c] = gs.reshape(NB, P).T

    block_rows = [P] * (NB - 1) + [HB - 48 * P]
    return dict(S128=S128, S256=S256, normc=normc, gsel=gsel,
                block_rows=block_rows, norm=norm)


def _build(S):
    KSTAGE = int(os.environ.get("KSTAGE", "99"))
    S1, S2 = S["S128"], S["S256"]
    NP1, NP2 = len(S1["pairs"]), len(S2["pairs"])
    TOTMX = max(S1["TOT"], S2["TOT"])
    NTA, NTB = NCORES * HA, NCORES * HB

    nc = bacc.Bacc(trn_type="TRN2", num_devices=NCORES,
                   dynamic_dma_scratch_size=32768, num_swdge_queues=4)

    sigA_in = nc.dram_tensor("sigA", [HA, 128], BF16, kind="ExternalInput")
    sigB_in = nc.dram_tensor("sigB", [HB, 128], BF16, kind="ExternalInput")
    idx1_in = nc.dram_tensor("idx128", [P, S1["TOT"] // 16], I16,
                             kind="ExternalInput")
    idx2_in = nc.dram_tensor("idx256", [P, S2["TOT"] // 16], I16,
                             kind="ExternalInput")
    ds1_in = nc.dram_tensor("dsel128", [P, NP1], F32, kind="ExternalInput")
    ds2_in = nc.dram_tensor("dsel256", [P, NP2], F32, kind="ExternalInput")
    normc_in = nc.dram_tensor("normc", [P, 4 * NB], F32, kind="ExternalInput")
    gsel_in = nc.dram_tensor("gsel", [P, NB], F32, kind="ExternalInput")
    iota16_in = nc.dram_tensor("iota16", [P, P], BF16, kind="ExternalInput")
    iotaf_in = nc.dram_tensor("iotaf", [P, P], F32, kind="ExternalInput")
    id16_in = nc.dram_tensor("id16", [P, P], BF16, kind="ExternalInput")
    idf_in = nc.dram_tensor("idf", [P, P], F32, kind="ExternalInput")
    w_in = [nc.dram_tensor(f"W{l}", [3 * DIN[l], DOUT[l]], BF16,
                           kind="ExternalInput") for l in range(4)]
    bt_in = [nc.dram_tensor(f"Bt{l}", [P, DOUT[l]], F32,
                            kind="ExternalInput") for l in range(4)]
    wm1_in = nc.dram_tensor("Wm1", [512, 512], BF16, kind="ExternalInput")
    bm1_in = nc.dram_tensor("Bm1", [P, 512], F32, kind="ExternalInput")
    wm2_in = nc.dram_tensor("Wm2", [512, 16], BF16, kind="ExternalInput")
    bm2_in = nc.dram_tensor("Bm2", [P, 16], F32, kind="ExternalInput")
    out = nc.dram_tensor("out", [P, 16], F32, kind="ExternalOutput")

    RG = [list(range(NCORES))]

    with tile.TileContext(nc) as tc:
        with (
            tc.tile_pool(name="dram", bufs=1, space="DRAM") as dram,
            tc.tile_pool(name="res", bufs=1) as res,
            tc.tile_pool(name="sb", bufs=2) as sb,
            tc.tile_pool(name="scp", bufs=1, space="PSUM") as scp,
            tc.tile_pool(name="pp", bufs=1, space="PSUM") as pp,
            tc.tile_pool(name="tpp", bufs=1, space="PSUM") as tpp,
            tc.tile_pool(name="rdp", bufs=1, space="PSUM") as rdp,
        ):
            # ---------- resident ----------
            idx_sb = res.tile([P, TOTMX // 16], I16, tag="idx")
            nc.sync.dma_start(out=idx_sb[:, : S1["TOT"] // 16],
                              in_=idx1_in[:, :])
            normc_sb = res.tile([P, 4 * NB], F32)
            gsel_sb = res.tile([P, NB], F32)
            iota16_sb = res.tile([P, P], BF16)
            iotaf_sb = res.tile([P, P], F32)
            id16_sb = res.tile([P, P], BF16)
            idf_sb = res.tile([P, P], F32)
            for t, src_t in ((normc_sb, normc_in), (gsel_sb, gsel_in),
                             (iota16_sb, iota16_in), (iotaf_sb, iotaf_in),
                             (id16_sb, id16_in), (idf_sb, idf_in)):
                nc.sync.dma_start(out=t[:], in_=src_t[:, :])
            w_sb = []
            for l in range(4):
                nchk = 3 * DIN[l] // P
                t = res.tile([P, nchk * DOUT[l]], BF16, tag=f"W{l}")
                for j in range(nchk):
                    nc.sync.dma_start(
                        out=t[:, j * DOUT[l] : (j + 1) * DOUT[l]],
                        in_=w_in[l][j * P : (j + 1) * P, :])
                w_sb.append(t)
            bt_sb = []
            for l in range(4):
                t = res.tile([P, DOUT[l]], F32, tag=f"Bt{l}")
                nc.sync.dma_start(out=t[:], in_=bt_in[l][:, :])
                bt_sb.append(t)
            wm1_sb = res.tile([P, 4 * 512], BF16)
            for j in range(4):
                nc.sync.dma_start(out=wm1_sb[:, j * 512 : (j + 1) * 512],
                                  in_=wm1_in[j * P : (j + 1) * P, :])
            bm1_sb = res.tile([P, 512], F32)
            nc.sync.dma_start(out=bm1_sb[:], in_=bm1_in[:, :])
            wm2_sb = res.tile([P, 4 * 16], BF16)
            for j in range(4):
                nc.sync.dma_start(out=wm2_sb[:, j * 16 : (j + 1) * 16],
                                  in_=wm2_in[j * P : (j + 1) * P, :])
            bm2_sb = res.tile([P, 16], F32)
            nc.sync.dma_start(out=bm2_sb[:], in_=bm2_in[:, :])

            # ---------- DRAM tiles ----------
            def dt(name, rows, d):
                return dram.tile([rows, d], BF16, tag=name, name=name,
                                 addr_space="Shared")

            # Shared tiles are single-writer: one table pair per (kind, layer)
            tabs = {}
            for l in range(3):
                tabs[("y0", 128, l)] = (dt(f"tA0_128_{l}", NTA, 128),
                                        dt(f"tB0_128_{l}", NTB, 128))
                tabs[("y1", 128, l)] = (dt(f"tA1_128_{l}", NTA, 128),
                                        dt(f"tB1_128_{l}", NTB, 128))
            tabs[("y0", 256, 3)] = (dt("tA0_256", NTA, 256),
                                    dt("tB0_256", NTB, 256))
            tabs[("y1", 256, 3)] = (dt("tA1_256", NTA, 256),
                                    dt("tB1_256", NTB, 256))
            # y0_128 shards are ping-ponged between layers so layer l+1's
            # writes don't serialize against layer l's block reads.
            shards = {
                ("y0", 128, 0): (dram.tile([HA, 128], BF16, name="shA0_128p0"),
                                 dram.tile([HB, 128], BF16, name="shB0_128p0")),
                ("y0", 128, 1): (dram.tile([HA, 128], BF16, name="shA0_128p1"),
                                 dram.tile([HB, 128], BF16, name="shB0_128p1")),
                ("y1", 128): (dram.tile([HA, 128], BF16, name="shA1_128"),
                              dram.tile([HB, 128], BF16, name="shB1_128")),
                ("y0", 256): (dram.tile([HA, 256], BF16, name="shA0_256"),
                              dram.tile([HB, 256], BF16, name="shB0_256")),
                ("y1", 256): (dram.tile([HA, 256], BF16, name="shA1_256"),
                              dram.tile([HB, 256], BF16, name="shB1_256")),
            }
            NQ1 = (NP1 + 3) // 4
            NQ2 = (NP2 + 3) // 4
            oh1_d = dram.tile([NQ1 * P, 4 * P], BF16, name="oh128")
            oh2_d = dram.tile([NQ2 * P, 4 * P], BF16, name="oh256")

            def ag_pair(shkey, tabkey):
                shA, shB = shards[shkey]
                tA, tB = tabs[tabkey]
                for sh, t in ((shA, tA), (shB, tB)):
                    nc.gpsimd.collective_compute(
                        "AllGather", mybir.AluOpType.bypass,
                        replica_groups=RG, ins=[sh[:]], outs=[t[:]])

            def ncol(kind, b):
                return normc_sb[:, kind * NB + b : kind * NB + b + 1]

            # ---------- one-hot cache build ----------
            ds_sb = res.tile([P, max(NP1, NP2)], F32, tag="dsel")
            nc.sync.dma_start(out=ds_sb[:, :NP1], in_=ds1_in[:, :])
            with nc.named_scope("ohbuild"):
                def build_class(NP, oh_d):
                    for grp in range(0, NP, 16):
                        n = min(16, NP - grp)
                        s = sb.tile([P, 16 * P], BF16, tag="ohb", bufs=3)
                        for jj in range(n):
                            nc.vector.tensor_scalar(
                                out=s[:, jj * P : (jj + 1) * P],
                                in0=iota16_sb[:],
                                scalar1=ds_sb[:, grp + jj : grp + jj + 1],
                                scalar2=None, op0=mybir.AluOpType.is_equal)
                        for jj in range(n):
                            j = grp + jj
                            q, sl = j // 4, j % 4
                            nc.sync.dma_start(
                                out=oh_d[q * P : (q + 1) * P,
                                         sl * P : (sl + 1) * P],
                                in_=s[:, jj * P : (jj + 1) * P])

                build_class(NP1, oh1_d)
                # class 256 dsel overwrites the same SBUF tile
                nc.sync.dma_start(out=ds_sb[:, :NP2], in_=ds2_in[:, :])
                build_class(NP2, oh2_d)

            # ---------- signal -> y0 tables ----------
            shA0, shB0 = shards[("y0", 128, 0)]
            nc.sync.dma_start(out=shA0[:, :], in_=sigA_in[:, :])
            nc.sync.dma_start(out=shB0[:, :], in_=sigB_in[:, :])
            ag_pair(("y0", 128, 0), ("y0", 128, 0))

            stopped = [KSTAGE == 0]

            def dump_and_stop(tab):
                d = sb.tile([P, 16], F32, tag="dmp", name="dmp")
                g = sb.tile([P, 16], BF16, tag="dmpg", name="dmpg")
                nc.sync.dma_start(out=g[:], in_=tab[0:P, 0:16])
                nc.vector.tensor_copy(out=d[:], in_=g[:])
                nc.sync.dma_start(out=out[:, :], in_=d[:])

            if stopped[0]:
                dump_and_stop(tabs[("y0", 128, 0)][0])

            # ---------- scatter unit ----------
            def scatter_unit(SS, tabpair, D, oh_d, evac_fn, uname):
                tA, tB = tabpair
                windows = [(tA, 0, CHUNK), (tA, CHUNK, NTA - CHUNK),
                           (tB, 0, CHUNK), (tB, CHUNK, NTB - CHUNK)]
                g_tiles = {}
                psums.clear()
                cur_q = [-1, None]
                emitted = -1
                with nc.named_scope(uname):
                    for j, (sidx, b) in enumerate(SS["pairs"]):
                        k = int(SS["sub_call"][sidx])
                        if k > emitted:
                            for kk in range(emitted + 1, k + 1):
                                w, o, L = SS["calls"][kk]
                                tab, tlo, trows = windows[w]
                                g = sb.tile([P, (MAX_CALL // P) * 256], BF16,
                                            tag="g", bufs=12, name="g")
                                nc.gpsimd.dma_gather(
                                    out_ap=g[:, : (L // P) * D].rearrange(
                                        "p (k d) -> p k d", d=D),
                                    in_ap=tab[tlo : tlo + trows, :],
                                    idxs_ap=idx_sb[:, o // 16 : (o + L) // 16],
                                    num_idxs=L,
                                    num_idxs_reg=L,
                                    elem_size=D,
                                    queue_num=kk % int(os.environ.get("KQ", "4")),
                                )
                                g_tiles[kk] = g
                            emitted = k
                        col = int(SS["sub_col"][sidx])
                        q, sl = j // 4, j % 4
                        if q != cur_q[0]:
                            ohq = sb.tile([P, 4 * P], BF16, tag="oh",
                                          bufs=12, name="oh")
                            eng = nc.sync if q % 2 == 0 else nc.scalar
                            eng.dma_start(
                                out=ohq[:], in_=oh_d[q * P : (q + 1) * P, :])
                            cur_q[0] = q
                            cur_q[1] = ohq
                        oh = cur_q[1][:, sl * P : (sl + 1) * P]
                        bk = b % SS["BB"]
                        acc_pb = 1  # one accumulation group per PSUM bank
                        bank = bk // acc_pb
                        cs = (bk % acc_pb) * D
                        bkey = (b // SS["BB"], bank)
                        if bkey not in psums:
                            psums[bkey] = scp.tile([P, 512], F32,
                                                   tag=f"scb{bank}",
                                                   name=f"scb{bank}")
                        nc.tensor.matmul(
                            out=psums[bkey][:, cs : cs + D],
                            lhsT=oh,
                            rhs=g_tiles[k][:, col * D : (col + 1) * D],
                            start=(j == SS["first_pair"][b]),
                            stop=(j == SS["last_pair"][b]),
                        )
                        if j == SS["last_pair"][b]:
                            evac_fn(b, psums[bkey][:, cs : cs + D])

            psums = {}

            def sh_dst(b):
                if b < HA_BLK:
                    return 0, b * P
                return 1, (b - HA_BLK) * P

            psum_r = rdp.tile([P, 512], F32, tag="rd")

            def evac_a(b, ps, width, l):
                rows = S["block_rows"][b]
                D = width
                half, off = sh_dst(b)
                sh = shards[("y1", width)][half]
                ev = sb.tile([P, 256], BF16, tag="ev", bufs=4)
                nc.any.tensor_scalar(
                    out=ev[:, :D], in0=ps, scalar1=ncol(0, b),
                    scalar2=None, op0=mybir.AluOpType.mult)
                nc.sync.dma_start(out=sh[off : off + rows, :],
                                  in_=ev[:rows, :D])
                if b == HA_BLK - 1:
                    nc.gpsimd.collective_compute(
                        "AllGather", mybir.AluOpType.bypass, replica_groups=RG,
                        ins=[shards[("y1", width)][0][:]],
                        outs=[tabs[("y1", width, l)][0][:]])
                if b == NB - 1:
                    nc.gpsimd.collective_compute(
                        "AllGather", mybir.AluOpType.bypass, replica_groups=RG,
                        ins=[shards[("y1", width)][1][:]],
                        outs=[tabs[("y1", width, l)][1][:]])

            def evac_b(b, ps, l):
                D = DIN[l]
                Do = DOUT[l]
                nkc = D // P
                rows = S["block_rows"][b]
                half, off = sh_dst(b)
                tb = sb.tile([P, 256], BF16, tag="tb", bufs=2)
                nc.any.tensor_scalar(
                    out=tb[:, :D], in0=ps, scalar1=ncol(1, b),
                    scalar2=None, op0=mybir.AluOpType.mult)
                y0b = sb.tile([P, 256], BF16, tag="y0b", bufs=2)
                y1b = sb.tile([P, 256], BF16, tag="y1b", bufs=2)
                if rows < P:
                    nc.any.memset(y0b[:], 0.0)
                    nc.any.memset(y1b[:], 0.0)
                y0key = ("y0", 256) if D == 256 else ("y0", 128, l % 2)
                nc.sync.dma_start(
                    out=y0b[:rows, :D],
                    in_=shards[y0key][half][off : off + rows, :])
                nc.sync.dma_start(
                    out=y1b[:rows, :D],
                    in_=shards[("y1", D)][half][off : off + rows, :])
                y2b = sb.tile([P, 256], BF16, tag="y2b", bufs=2)
                nc.any.tensor_tensor(
                    out=y2b[:, :D], in0=tb[:, :D], in1=y0b[:, :D],
                    op=mybir.AluOpType.subtract)
                yts = []
                for term, ysrc in enumerate((y0b, y1b, y2b)):
                    for kc in range(nkc):
                        tp = tpp.tile([P, P], BF16, tag="tp16", bufs=2)
                        nc.tensor.transpose(
                            out=tp[:], in_=ysrc[:, kc * P : (kc + 1) * P],
                            identity=id16_sb[:])
                        yt = sb.tile([P, P], BF16, tag="yt", bufs=8, name="yt")
                        nc.vector.tensor_copy(out=yt[:], in_=tp[:])
                        yts.append(yt)
                ph = pp.tile([P, 512], F32, tag="dh")
                nchk = 3 * nkc
                for j2 in range(nchk):
                    nc.tensor.matmul(
                        out=ph[:, :Do], lhsT=yts[j2][:],
                        rhs=w_sb[l][:, j2 * Do : (j2 + 1) * Do],
                        start=(j2 == 0), stop=(j2 == nchk - 1))
                t1 = sb.tile([P, 512], F32, tag="t1", bufs=2)
                nc.any.tensor_scalar(
                    out=t1[:, :Do], in0=ph[:, :Do], scalar1=ncol(2, b),
                    scalar2=None, op0=mybir.AluOpType.mult)
                t2 = sb.tile([P, 512], F32, tag="t2", bufs=2)
                nc.any.tensor_tensor(
                    out=t2[:, :Do], in0=t1[:, :Do], in1=bt_sb[l][:],
                    op=mybir.AluOpType.add)
                if l < 3:
                    wo = DOUT[l]
                    o_t = sb.tile([P, 256], BF16, tag="lo", bufs=2)
                    nc.scalar.activation(
                        o_t[:, :wo], t2[:, :wo],
                        mybir.ActivationFunctionType.Relu,
                        scale=ncol(3, b))
                    wkey = (("y0", 256) if wo >= 256
                            else ("y0", 128, (l + 1) % 2))
                    wclass = 256 if wo >= 256 else 128
                    sh = shards[wkey][half]
                    nc.sync.dma_start(out=sh[off : off + rows, :],
                                      in_=o_t[:rows, :wo])
                    if b == HA_BLK - 1:
                        nc.gpsimd.collective_compute(
                            "AllGather", mybir.AluOpType.bypass,
                            replica_groups=RG,
                            ins=[shards[wkey][0][:]],
                            outs=[tabs[("y0", wclass, l + 1)][0][:]])
                    if b == NB - 1:
                        nc.gpsimd.collective_compute(
                            "AllGather", mybir.AluOpType.bypass,
                            replica_groups=RG,
                            ins=[shards[wkey][1][:]],
                            outs=[tabs[("y0", wclass, l + 1)][1][:]])
                else:
                    h = sb.tile([P, 512], BF16, tag="h", bufs=2)
                    nc.scalar.activation(
                        h[:], t2[:], mybir.ActivationFunctionType.Relu)
                    gs = sb.tile([P, P], BF16, tag="gs", bufs=2)
                    nc.vector.tensor_scalar(
                        out=gs[:], in0=iota16_sb[:],
                        scalar1=gsel_sb[:, b : b + 1], scalar2=None,
                        op0=mybir.AluOpType.is_equal)
                    nc.tensor.matmul(
                        out=psum_r[:], lhsT=gs[:], rhs=h[:],
                        start=(b == 0), stop=(b == NB - 1))

            # ---------- layers ----------
            for l in range(4):
                if stopped[0]:
                    break
                width = DIN[l]
                SS = S["S128"] if width == 128 else S["S256"]
                oh_d = oh1_d if width == 128 else oh2_d
                if l == 3:
                    # swap in the 256-class indices
                    nc.sync.dma_start(out=idx_sb[:, : S2["TOT"] // 16],
                                      in_=idx2_in[:, :])
                scatter_unit(SS, tabs[("y0", width, l)], width, oh_d,
                             lambda b, ps, wd=width, ll=l:
                                 evac_a(b, ps, wd, ll),
                             f"u{l}A")
                if KSTAGE == 10 + l:
                    dump_and_stop(tabs[("y1", width, l)][0])
                    stopped[0] = True
                    break
                scatter_unit(SS, tabs[("y1", width, l)], width, oh_d,
                             lambda b, ps, ll=l: evac_b(b, ps, ll),
                             f"u{l}B")
                if KSTAGE == 20 + l:
                    if l < 3:
                        dump_and_stop(
                            tabs[("y0", 256 if DOUT[l] >= 256 else 128,
                                  l + 1)][0])
                    stopped[0] = True
                    break

            # ---------- readout + MLP ----------
            if not stopped[0]:
                hgp = sb.tile([P, 512], F32, tag="hgp")
                nc.any.tensor_copy(out=hgp[:], in_=psum_r[:])
                part_d = dram.tile([P, 512], F32, tag="part", name="part")
                tot_d = dram.tile([P, 512], F32, tag="tot", name="tot",
                                  addr_space="Shared")
                nc.sync.dma_start(out=part_d[:], in_=hgp[:])
                nc.gpsimd.collective_compute(
                    "AllReduce", mybir.AluOpType.add,
                    replica_groups=RG, ins=[part_d[:]], outs=[tot_d[:]])
                hg = sb.tile([P, 512], F32, tag="hg")
                nc.sync.dma_start(out=hg[:], in_=tot_d[:])

                def mm_block(x_sb, wtile, dout, bias_tile, relu):
                    xts = []
                    for kc in range(4):
                        tp = tpp.tile([P, P], F32, tag="tpf", bufs=1)
                        nc.tensor.transpose(
                            out=tp[:], in_=x_sb[:, kc * P : (kc + 1) * P],
                            identity=idf_sb[:])
                        xt = sb.tile([P, P], BF16, tag="yt", bufs=8, name="xt")
                        nc.any.tensor_copy(out=xt[:], in_=tp[:])
                        xts.append(xt)
                    ph = pp.tile([P, 512], F32, tag="dh")
                    for kc in range(4):
                        nc.tensor.matmul(
                            out=ph[:, :dout], lhsT=xts[kc][:],
                            rhs=wtile[:, kc * dout : (kc + 1) * dout],
                            start=(kc == 0), stop=(kc == 3))
                    o_t = sb.tile([P, dout], F32, tag=f"mo{dout}")
                    nc.any.tensor_tensor(
                        out=o_t[:], in0=ph[:, :dout], in1=bias_tile[:],
                        op=mybir.AluOpType.add)
                    if relu:
                        r = sb.tile([P, dout], F32, tag=f"mr{dout}")
                        nc.scalar.activation(
                            r[:], o_t[:], mybir.ActivationFunctionType.Relu)
                        return r
                    return o_t

                m1 = mm_block(hg, wm1_sb, 512, bm1_sb, relu=True)
                m2 = mm_block(m1, wm2_sb, 16, bm2_sb, relu=False)
                nc.sync.dma_start(out=out[:, :], in_=m2[:])

    nc.finalize()
    return nc


_CACHE = {}
G_OVERRIDE = None
TRACE = False
LAST_RESULTS = None


def kernel(signal, W0, b0, W1, b1, W2, b2, W3, b3, Wm1, bm1, Wm2, bm2,
           src, dst, graph_ids):
    global LAST_RESULTS
    import ml_dtypes

    signal = np.ascontiguousarray(np.asarray(signal, dtype=np.float32))
    src = np.asarray(src).astype(np.int64)
    dst = np.asarray(dst).astype(np.int64)
    graph_ids = np.asarray(graph_ids).astype(np.int64)
    N = signal.shape[0]
    G = G_OVERRIDE or 128

    key = (N, src.shape[0], G, hash(src.tobytes()) ^ hash(dst.tobytes())
           ^ hash(graph_ids.tobytes()))
    if key in _CACHE:
        S, nc = _CACHE[key]
    else:
        S = _preprocess(src, dst, graph_ids, N, G)
        nc = _build(S)
        _CACHE.clear()
        _CACHE[key] = (S, nc)

    S1, S2 = S["S128"], S["S256"]
    y0 = signal * S["norm"][:, None]
    iota_np = np.broadcast_to(np.arange(P, dtype=np.float32)[None, :],
                              (P, P)).copy()
    ident_np = np.eye(P, dtype=np.float32)
    ws = [np.asarray(w, dtype=np.float32).astype(ml_dtypes.bfloat16)
          for w in (W0, W1, W2, W3)]
    bts = [np.broadcast_to(np.asarray(b, np.float32)[None, :],
                           (P, len(b))).copy() for b in (b0, b1, b2, b3)]
    wm2_p = np.zeros((512, 16), np.float32)
    wm2_p[:, :10] = np.asarray(Wm2, dtype=np.float32)
    bm2_p = np.zeros((P, 16), np.float32)
    bm2_p[:, :10] = np.asarray(bm2, dtype=np.float32)[None, :]
    bm1_t = np.broadcast_to(np.asarray(bm1, np.float32)[None, :],
                            (P, 512)).copy()

    in_maps = []
    for c in range(NCORES):
        lo = c * NLOC
        m = {
            "sigA": y0[lo : lo + HA].astype(ml_dtypes.bfloat16),
            "sigB": y0[lo + HA : lo + NLOC].astype(ml_dtypes.bfloat16),
            "idx128": S1["idx16"][c],
            "idx256": S2["idx16"][c],
            "dsel128": S1["dsel"][c],
            "dsel256": S2["dsel"][c],
            "normc": S["normc"][c],
            "gsel": S["gsel"][c],
            "iota16": iota_np.astype(ml_dtypes.bfloat16),
            "iotaf": iota_np,
            "id16": ident_np.astype(ml_dtypes.bfloat16),
            "idf": ident_np,
            "Wm1": np.asarray(Wm1, np.float32).astype(ml_dtypes.bfloat16),
            "Bm1": bm1_t,
            "Wm2": wm2_p.astype(ml_dtypes.bfloat16),
            "Bm2": bm2_p,
        }
        for l in range(4):
            m[f"W{l}"] = ws[l]
            m[f"Bt{l}"] = bts[l]
        in_maps.append(m)

    res = run_bass_kernel_spmd(
        nc, in_maps, core_ids=list(range(NCORES)), trace=TRACE
    )
    LAST_RESULTS = res
    return np.asarray(res.results[0]["out"][:G, :10])


# revision 17
# speedup vs baseline: 2.1046x; 1.2599x over previous
"""Trainium2 Bass kernel for the DGL-style ChebConv GNN classifier (v2).

Strategy (8 NeuronCores, SPMD):
  - Nodes sharded contiguously (12.5K/core), split into half-shards A (6272
    rows = 49 blocks) and B (6228 rows) so table AllGathers pipeline with
    compute at half-shard granularity.
  - All tables, gathers and matmuls in bf16 (PSUM accumulation fp32).
  - Per sparse pass: dma_gather (bf16 rows, 1024-idx calls, 4 SWDGE queues
    round-robin with a deep tile pool so the 4 Q7 pairs generate descriptors
    concurrently), then segment-sum via one-hot matmuls into per-block PSUM.
  - One-hot matrices are built once on DVE (is_equal vs iota) and cached in
    DRAM; each pass streams them back via HWDGE, freeing DVE.
  - Dense (concat @ W) matmuls consume PE-transposed bf16 blocks.
  - Readout via per-core graph-id one-hot matmul partial sums + fp32
    AllReduce, then the small MLP classifier on-chip.

kernel(**inputs) takes FULL unsharded inputs, returns FULL [G, 10] output.
"""

import os

import numpy as np

import concourse.bacc as bacc
import concourse.mybir as mybir
import concourse.tile as tile
from concourse.bass_utils import run_bass_kernel_spmd

NCORES = 8
P = 128
NLOC = 12500
NB = 98               # blocks per core
HA_BLK = 49
HA = HA_BLK * P       # 6272 rows, half A
HB = NLOC - HA        # 6228 rows, half B
CHUNK = 32768
MAX_CALL = 1024
F32 = mybir.dt.float32
BF16 = mybir.dt.bfloat16
I16 = mybir.dt.int16

DIN = [128, 128, 128, 256]
DOUT = [128, 128, 256, 512]
BB128 = int(os.environ.get("KBB128", "3"))
BB256 = int(os.environ.get("KBB256", "3"))


def _wrap16(local_idx):
    L = local_idx.shape[0]
    w = local_idx.reshape(L // 16, 16).T.copy()
    return np.tile(w, (8, 1))


def _node_map(N):
    """node id -> (window 0..3, idx within window) over the reindexed tables.

    tabA rows: core c half-A local i (<HA) at c*HA+i; windows [0,32768),
    [32768,50176). tabB rows: c*HB + (i-HA); windows [0,32768), [32768,49824).
    """
    v = np.arange(N, dtype=np.int64)
    c = v // NLOC
    i = v % NLOC
    in_a = i < HA
    trow = np.where(in_a, c * HA + i, c * HB + (i - HA))
    hi = trow >= CHUNK
    win = np.where(in_a, 0, 2) + hi
    widx = trow - hi * CHUNK
    return win, widx


def _edge_struct(src, dst, BB):
    """Per-width-class slot/call/pair structure (shared across cores)."""
    win, widx = _node_map(100000)
    NBATCH = (NB + BB - 1) // BB
    NW = 4
    per_core = []
    counts = np.zeros((NCORES, NBATCH, NW), dtype=np.int64)
    core_of = dst // NLOC
    for c in range(NCORES):
        m = core_of == c
        s = src[m]
        dl = dst[m] - c * NLOC
        bat = (dl // P) // BB
        w = win[s]
        order = np.lexsort((dl, w, bat))
        s, dl, bat, w = s[order], dl[order], bat[order], w[order]
        key = bat * NW + w
        counts[c] = np.bincount(key, minlength=NBATCH * NW).reshape(NBATCH, NW)
        per_core.append((s, dl, key))

    runlen = ((counts.max(axis=0) + P - 1) // P) * P     # [NBATCH, NW]
    run_off = np.zeros((NBATCH, NW), dtype=np.int64)
    tot = 0
    for t in range(NBATCH):
        for w in range(NW):
            run_off[t, w] = tot
            tot += runlen[t, w]
    TOT = int(tot)
    TOT = ((TOT + 1023) // 1024) * 1024   # keep idx buffer 16-col aligned
    NSUB = TOT // P

    slot_widx = np.zeros((NCORES, TOT), dtype=np.int64)
    slot_dstl = np.full((NCORES, TOT), -1, dtype=np.int64)
    slot_win = np.zeros(TOT, dtype=np.int64)
    for t in range(NBATCH):
        for w in range(NW):
            o, L = run_off[t, w], runlen[t, w]
            slot_win[o : o + L] = w
    for c in range(NCORES):
        s, dl, key = per_core[c]
        kcnt = counts[c].reshape(-1)
        koff = run_off.reshape(-1)
        pos = np.empty(len(key), dtype=np.int64)
        start = 0
        for k in range(NBATCH * NW):
            n = kcnt[k]
            pos[start : start + n] = koff[k] + np.arange(n)
            start += n
        slot_widx[c][pos] = widx[s]
        slot_dstl[c][pos] = dl

    calls = []      # (window, slot_off, len)
    for t in range(NBATCH):
        for w in range(NW):
            o, L = int(run_off[t, w]), int(runlen[t, w])
            while L > 0:
                seg = min(L, MAX_CALL)
                calls.append((w, o, seg))
                o += seg
                L -= seg

    idx16 = np.zeros((NCORES, P, TOT // 16), dtype=np.int16)
    for c in range(NCORES):
        for w, o, L in calls:
            local = slot_widx[c, o : o + L].astype(np.int16)
            idx16[c][:, o // 16 : (o + L) // 16] = _wrap16(local)

    sub_call = np.zeros(NSUB, dtype=np.int64)
    sub_col = np.zeros(NSUB, dtype=np.int64)
    for k, (w, o, L) in enumerate(calls):
        for j in range(L // P):
            sub_call[o // P + j] = k
            sub_col[o // P + j] = j

    blk_all = np.where(slot_dstl >= 0, slot_dstl // P, -1)
    pairs = []
    for sidx in range(NSUB):
        sl = blk_all[:, sidx * P : (sidx + 1) * P]
        present = np.unique(sl[sl >= 0])
        for b in present:
            pairs.append((sidx, int(b)))
    NPAIRS = len(pairs)

    first_pair = {}
    last_pair = {}
    for j, (sidx, b) in enumerate(pairs):
        if b not in first_pair:
            first_pair[b] = j
        last_pair[b] = j

    dsel = np.full((NCORES, P, NPAIRS), -1.0, dtype=np.float32)
    for j, (sidx, b) in enumerate(pairs):
        sl = slot_dstl[:, sidx * P : (sidx + 1) * P]
        m = (sl // P) == b
        dsel[:, :, j] = np.where(m, (sl - b * P).astype(np.float32), -1.0)

    return dict(
        BB=BB, NBATCH=NBATCH, TOT=TOT, NSUB=NSUB, calls=calls,
        pairs=pairs, first_pair=first_pair, last_pair=last_pair,
        sub_call=sub_call, sub_col=sub_col, idx16=idx16, dsel=dsel,
    )


def _preprocess(src, dst, graph_ids, N, G):
    assert N == NCORES * NLOC
    deg = np.bincount(dst, minlength=N).astype(np.float32)
    norm = np.clip(deg, 1.0, None) ** -0.5
    norm2 = norm * norm
    inv_norm = 1.0 / norm

    S128 = _edge_struct(src, dst, BB128)
    S256 = _edge_struct(src, dst, BB256)

    normc = np.zeros((NCORES, P, 4 * NB), dtype=np.float32)
    gsel = np.full((NCORES, P, NB), -1.0, dtype=np.float32)
    for c in range(NCORES):
        lo = c * NLOC
        hi = lo + NLOC
        pad = NB * P - NLOC
        nn = np.pad(norm[lo:hi], (0, pad)).reshape(NB, P).T
        n2 = np.pad(norm2[lo:hi], (0, pad)).reshape(NB, P).T
        iv = np.pad(inv_norm[lo:hi], (0, pad)).reshape(NB, P).T
        normc[c][:, 0 * NB : 1 * NB] = -n2
        normc[c][:, 1 * NB : 2 * NB] = -2.0 * n2
        normc[c][:, 2 * NB : 3 * NB] = iv
        normc[c][:, 3 * NB : 4 * NB] = nn
        gs = np.pad(graph_ids[lo:hi].astype(np.float32), (0, pad),
                    constant_values=-1.0)
        gsel[c] = gs.reshape(NB, P).T

    block_rows = [P] * (NB - 1) + [HB - 48 * P]
    return dict(S128=S128, S256=S256, normc=normc, gsel=gsel,
                block_rows=block_rows, norm=norm)


def _build(S):
    KSTAGE = int(os.environ.get("KSTAGE", "99"))
    S1, S2 = S["S128"], S["S256"]
    NP1, NP2 = len(S1["pairs"]), len(S2["pairs"])
    TOTMX = max(S1["TOT"], S2["TOT"])
    NTA, NTB = NCORES * HA, NCORES * HB

    nc = bacc.Bacc(trn_type="TRN2", num_devices=NCORES,
                   dynamic_dma_scratch_size=32768, num_swdge_queues=4)

    sigA_in = nc.dram_tensor("sigA", [HA, 128], BF16, kind="ExternalInput")
    sigB_in = nc.dram_tensor("sigB", [HB, 128], BF16, kind="ExternalInput")
    idx1_in = nc.dram_tensor("idx128", [P, S1["TOT"] // 16], I16,
                             kind="ExternalInput")
    idx2_in = nc.dram_tensor("idx256", [P, S2["TOT"] // 16], I16,
                             kind="ExternalInput")
    ds1_in = nc.dram_tensor("dsel128", [P, NP1], F32, kind="ExternalInput")
    ds2_in = nc.dram_tensor("dsel256", [P, NP2], F32, kind="ExternalInput")
    normc_in = nc.dram_tensor("normc", [P, 4 * NB], F32, kind="ExternalInput")
    gsel_in = nc.dram_tensor("gsel", [P, NB], F32, kind="ExternalInput")
    iota16_in = nc.dram_tensor("iota16", [P, P], BF16, kind="ExternalInput")
    iotaf_in = nc.dram_tensor("iotaf", [P, P], F32, kind="ExternalInput")
    id16_in = nc.dram_tensor("id16", [P, P], BF16, kind="ExternalInput")
    idf_in = nc.dram_tensor("idf", [P, P], F32, kind="ExternalInput")
    w_in = [nc.dram_tensor(f"W{l}", [3 * DIN[l], DOUT[l]], BF16,
                           kind="ExternalInput") for l in range(4)]
    bt_in = [nc.dram_tensor(f"Bt{l}", [P, DOUT[l]], F32,
                            kind="ExternalInput") for l in range(4)]
    wm1_in = nc.dram_tensor("Wm1", [512, 512], BF16, kind="ExternalInput")
    bm1_in = nc.dram_tensor("Bm1", [P, 512], F32, kind="ExternalInput")
    wm2_in = nc.dram_tensor("Wm2", [512, 16], BF16, kind="ExternalInput")
    bm2_in = nc.dram_tensor("Bm2", [P, 16], F32, kind="ExternalInput")
    out = nc.dram_tensor("out", [P, 16], F32, kind="ExternalOutput")

    RG = [list(range(NCORES))]

    with tile.TileContext(nc) as tc:
        with (
            tc.tile_pool(name="dram", bufs=1, space="DRAM") as dram,
            tc.tile_pool(name="res", bufs=1) as res,
            tc.tile_pool(name="sb", bufs=2) as sb,
            tc.tile_pool(name="scp", bufs=1, space="PSUM") as scp,
            tc.tile_pool(name="pp", bufs=1, space="PSUM") as pp,
            tc.tile_pool(name="tpp", bufs=1, space="PSUM") as tpp,
            tc.tile_pool(name="rdp", bufs=1, space="PSUM") as rdp,
        ):
            # ---------- resident ----------
            idx_sb = res.tile([P, TOTMX // 16], I16, tag="idx")
            nc.sync.dma_start(out=idx_sb[:, : S1["TOT"] // 16],
                              in_=idx1_in[:, :])
            normc_sb = res.tile([P, 4 * NB], F32)
            gsel_sb = res.tile([P, NB], F32)
            iota16_sb = res.tile([P, P], BF16)
            iotaf_sb = res.tile([P, P], F32)
            id16_sb = res.tile([P, P], BF16)
            idf_sb = res.tile([P, P], F32)
            for t, src_t in ((normc_sb, normc_in), (gsel_sb, gsel_in),
                             (iota16_sb, iota16_in), (iotaf_sb, iotaf_in),
                             (id16_sb, id16_in), (idf_sb, idf_in)):
                nc.sync.dma_start(out=t[:], in_=src_t[:, :])
            w_sb = []
            for l in range(4):
                nchk = 3 * DIN[l] // P
                t = res.tile([P, nchk * DOUT[l]], BF16, tag=f"W{l}")
                for j in range(nchk):
                    nc.sync.dma_start(
                        out=t[:, j * DOUT[l] : (j + 1) * DOUT[l]],
                        in_=w_in[l][j * P : (j + 1) * P, :])
                w_sb.append(t)
            bt_sb = []
            for l in range(4):
                t = res.tile([P, DOUT[l]], F32, tag=f"Bt{l}")
                nc.sync.dma_start(out=t[:], in_=bt_in[l][:, :])
                bt_sb.append(t)
            wm1_sb = res.tile([P, 4 * 512], BF16)
            for j in range(4):
                nc.sync.dma_start(out=wm1_sb[:, j * 512 : (j + 1) * 512],
                                  in_=wm1_in[j * P : (j + 1) * P, :])
            bm1_sb = res.tile([P, 512], F32)
            nc.sync.dma_start(out=bm1_sb[:], in_=bm1_in[:, :])
            wm2_sb = res.tile([P, 4 * 16], BF16)
            for j in range(4):
                nc.sync.dma_start(out=wm2_sb[:, j * 16 : (j + 1) * 16],
                                  in_=wm2_in[j * P : (j + 1) * P, :])
            bm2_sb = res.tile([P, 16], F32)
            nc.sync.dma_start(out=bm2_sb[:], in_=bm2_in[:, :])

            # ---------- DRAM tiles ----------
            def dt(name, rows, d):
                return dram.tile([rows, d], BF16, tag=name, name=name,
                                 addr_space="Shared")

            # Shared tiles are single-writer: one table pair per (kind, layer)
            tabs = {}
            for l in range(3):
                tabs[("y0", 128, l)] = (dt(f"tA0_128_{l}", NTA, 128),
                                        dt(f"tB0_128_{l}", NTB, 128))
                tabs[("y1", 128, l)] = (dt(f"tA1_128_{l}", NTA, 128),
                                        dt(f"tB1_128_{l}", NTB, 128))
            tabs[("y0", 256, 3)] = (dt("tA0_256", NTA, 256),
                                    dt("tB0_256", NTB, 256))
            tabs[("y1", 256, 3)] = (dt("tA1_256", NTA, 256),
                                    dt("tB1_256", NTB, 256))
            # y0_128 shards are ping-ponged between layers so layer l+1's
            # writes don't serialize against layer l's block reads.
            shards = {
                ("y0", 128, 0): (dram.tile([HA, 128], BF16, name="shA0_128p0"),
                                 dram.tile([HB, 128], BF16, name="shB0_128p0")),
                ("y0", 128, 1): (dram.tile([HA, 128], BF16, name="shA0_128p1"),
                                 dram.tile([HB, 128], BF16, name="shB0_128p1")),
                ("y1", 128): (dram.tile([HA, 128], BF16, name="shA1_128"),
                              dram.tile([HB, 128], BF16, name="shB1_128")),
                ("y0", 256): (dram.tile([HA, 256], BF16, name="shA0_256"),
                              dram.tile([HB, 256], BF16, name="shB0_256")),
                ("y1", 256): (dram.tile([HA, 256], BF16, name="shA1_256"),
                              dram.tile([HB, 256], BF16, name="shB1_256")),
            }
            NO1 = (NP1 + 7) // 8
            NO2 = (NP2 + 7) // 8
            NO1L = NO1 // 2
            NO2L = NO2 // 2
            oh1_d = (dram.tile([NO1L * P, 8 * P], BF16, name="oh128a"),
                     dram.tile([(NO1 - NO1L) * P, 8 * P], BF16, name="oh128b"))
            oh2_d = (dram.tile([NO2L * P, 8 * P], BF16, name="oh256a"),
                     dram.tile([(NO2 - NO2L) * P, 8 * P], BF16, name="oh256b"))

            def ag_pair(shkey, tabkey):
                shA, shB = shards[shkey]
                tA, tB = tabs[tabkey]
                for sh, t in ((shA, tA), (shB, tB)):
                    nc.gpsimd.collective_compute(
                        "AllGather", mybir.AluOpType.bypass,
                        replica_groups=RG, ins=[sh[:]], outs=[t[:]])

            def ncol(kind, b):
                return normc_sb[:, kind * NB + b : kind * NB + b + 1]

            # ---------- one-hot cache build ----------
            ds_sb = res.tile([P, max(NP1, NP2)], F32, tag="dsel")
            nc.sync.dma_start(out=ds_sb[:, :NP1], in_=ds1_in[:, :])
            with nc.named_scope("ohbuild"):
                def build_class(NP, oh_d, NOL):
                    for grp in range(0, NP, 8):
                        n = min(8, NP - grp)
                        s = sb.tile([P, 8 * P], BF16, tag="ohb", bufs=4)
                        for jj in range(n):
                            nc.vector.tensor_scalar(
                                out=s[:, jj * P : (jj + 1) * P],
                                in0=iota16_sb[:],
                                scalar1=ds_sb[:, grp + jj : grp + jj + 1],
                                scalar2=None, op0=mybir.AluOpType.is_equal)
                        o = grp // 8
                        tgt, ob = (oh_d[0], o) if o < NOL else (oh_d[1], o - NOL)
                        eng = nc.scalar if o % 2 == 0 else nc.sync
                        eng.dma_start(out=tgt[ob * P : (ob + 1) * P, :],
                                      in_=s[:])

                build_class(NP1, oh1_d, NO1L)
                # class 256 dsel overwrites the same SBUF tile
                nc.sync.dma_start(out=ds_sb[:, :NP2], in_=ds2_in[:, :])
                build_class(NP2, oh2_d, NO2L)

            # ---------- signal -> y0 tables ----------
            shA0, shB0 = shards[("y0", 128, 0)]
            nc.sync.dma_start(out=shA0[:, :], in_=sigA_in[:, :])
            nc.sync.dma_start(out=shB0[:, :], in_=sigB_in[:, :])
            ag_pair(("y0", 128, 0), ("y0", 128, 0))

            stopped = [KSTAGE == 0]

            def dump_and_stop(tab):
                d = sb.tile([P, 16], F32, tag="dmp", name="dmp")
                g = sb.tile([P, 16], BF16, tag="dmpg", name="dmpg")
                nc.sync.dma_start(out=g[:], in_=tab[0:P, 0:16])
                nc.vector.tensor_copy(out=d[:], in_=g[:])
                nc.sync.dma_start(out=out[:, :], in_=d[:])

            if stopped[0]:
                dump_and_stop(tabs[("y0", 128, 0)][0])

            # ---------- scatter unit ----------
            def scatter_unit(SS, tabpair, D, oh_d, oh_nol, evac_fn, uname):
                tA, tB = tabpair
                windows = [(tA, 0, CHUNK), (tA, CHUNK, NTA - CHUNK),
                           (tB, 0, CHUNK), (tB, CHUNK, NTB - CHUNK)]
                g_tiles = {}
                psums.clear()
                cur_q = [-1, None]
                emitted = -1
                with nc.named_scope(uname):
                    for j, (sidx, b) in enumerate(SS["pairs"]):
                        k = int(SS["sub_call"][sidx])
                        if k > emitted:
                            for kk in range(emitted + 1, k + 1):
                                w, o, L = SS["calls"][kk]
                                tab, tlo, trows = windows[w]
                                g = sb.tile([P, (MAX_CALL // P) * 256], BF16,
                                            tag="g", bufs=14, name="g")
                                nc.gpsimd.dma_gather(
                                    out_ap=g[:, : (L // P) * D].rearrange(
                                        "p (k d) -> p k d", d=D),
                                    in_ap=tab[tlo : tlo + trows, :],
                                    idxs_ap=idx_sb[:, o // 16 : (o + L) // 16],
                                    num_idxs=L,
                                    num_idxs_reg=L,
                                    elem_size=D,
                                    queue_num=kk % int(os.environ.get("KQ", "4")),
                                )
                                g_tiles[kk] = g
                            emitted = k
                        col = int(SS["sub_col"][sidx])
                        q, sl = j // 8, j % 8
                        if q != cur_q[0]:
                            ohq = sb.tile([P, 8 * P], BF16, tag="oh",
                                          bufs=8, name="oh")
                            tgt, ob = ((oh_d[0], q) if q < oh_nol
                                       else (oh_d[1], q - oh_nol))
                            eng = nc.sync if q % 2 == 0 else nc.scalar
                            eng.dma_start(
                                out=ohq[:], in_=tgt[ob * P : (ob + 1) * P, :])
                            cur_q[0] = q
                            cur_q[1] = ohq
                        oh = cur_q[1][:, sl * P : (sl + 1) * P]
                        bk = b % SS["BB"]
                        acc_pb = 1  # one accumulation group per PSUM bank
                        bank = bk // acc_pb
                        cs = (bk % acc_pb) * D
                        bkey = (b // SS["BB"], bank)
                        if bkey not in psums:
                            psums[bkey] = scp.tile([P, 512], F32,
                                                   tag=f"scb{bank}",
                                                   name=f"scb{bank}")
                        nc.tensor.matmul(
                            out=psums[bkey][:, cs : cs + D],
                            lhsT=oh,
                            rhs=g_tiles[k][:, col * D : (col + 1) * D],
                            start=(j == SS["first_pair"][b]),
                            stop=(j == SS["last_pair"][b]),
                        )
                        if j == SS["last_pair"][b]:
                            evac_fn(b, psums[bkey][:, cs : cs + D])

            psums = {}

            def sh_dst(b):
                if b < HA_BLK:
                    return 0, b * P
                return 1, (b - HA_BLK) * P

            psum_r = rdp.tile([P, 512], F32, tag="rd")

            def evac_a(b, ps, width, l):
                rows = S["block_rows"][b]
                D = width
                half, off = sh_dst(b)
                sh = shards[("y1", width)][half]
                ev = sb.tile([P, 256], BF16, tag="ev", bufs=4)
                nc.any.tensor_scalar(
                    out=ev[:, :D], in0=ps, scalar1=ncol(0, b),
                    scalar2=None, op0=mybir.AluOpType.mult)
                nc.sync.dma_start(out=sh[off : off + rows, :],
                                  in_=ev[:rows, :D])
                if b == HA_BLK - 1:
                    nc.gpsimd.collective_compute(
                        "AllGather", mybir.AluOpType.bypass, replica_groups=RG,
                        ins=[shards[("y1", width)][0][:]],
                        outs=[tabs[("y1", width, l)][0][:]])
                if b == NB - 1:
                    nc.gpsimd.collective_compute(
                        "AllGather", mybir.AluOpType.bypass, replica_groups=RG,
                        ins=[shards[("y1", width)][1][:]],
                        outs=[tabs[("y1", width, l)][1][:]])

            def evac_b(b, ps, l):
                D = DIN[l]
                Do = DOUT[l]
                nkc = D // P
                rows = S["block_rows"][b]
                half, off = sh_dst(b)
                tb = sb.tile([P, 256], BF16, tag="tb", bufs=2)
                nc.any.tensor_scalar(
                    out=tb[:, :D], in0=ps, scalar1=ncol(1, b),
                    scalar2=None, op0=mybir.AluOpType.mult)
                y0b = sb.tile([P, 256], BF16, tag="y0b", bufs=2)
                y1b = sb.tile([P, 256], BF16, tag="y1b", bufs=2)
                if rows < P:
                    nc.any.memset(y0b[:], 0.0)
                    nc.any.memset(y1b[:], 0.0)
                y0key = ("y0", 256) if D == 256 else ("y0", 128, l % 2)
                nc.sync.dma_start(
                    out=y0b[:rows, :D],
                    in_=shards[y0key][half][off : off + rows, :])
                nc.sync.dma_start(
                    out=y1b[:rows, :D],
                    in_=shards[("y1", D)][half][off : off + rows, :])
                y2b = sb.tile([P, 256], BF16, tag="y2b", bufs=2)
                nc.any.tensor_tensor(
                    out=y2b[:, :D], in0=tb[:, :D], in1=y0b[:, :D],
                    op=mybir.AluOpType.subtract)
                yts = []
                for term, ysrc in enumerate((y0b, y1b, y2b)):
                    for kc in range(nkc):
                        tp = tpp.tile([P, P], BF16, tag="tp16", bufs=2)
                        nc.tensor.transpose(
                            out=tp[:], in_=ysrc[:, kc * P : (kc + 1) * P],
                            identity=id16_sb[:])
                        yt = sb.tile([P, P], BF16, tag="yt", bufs=8, name="yt")
                        nc.vector.tensor_copy(out=yt[:], in_=tp[:])
                        yts.append(yt)
                ph = pp.tile([P, 512], F32, tag="dh")
                nchk = 3 * nkc
                for j2 in range(nchk):
                    nc.tensor.matmul(
                        out=ph[:, :Do], lhsT=yts[j2][:],
                        rhs=w_sb[l][:, j2 * Do : (j2 + 1) * Do],
                        start=(j2 == 0), stop=(j2 == nchk - 1))
                t1 = sb.tile([P, 512], F32, tag="t1", bufs=2)
                nc.any.tensor_scalar(
                    out=t1[:, :Do], in0=ph[:, :Do], scalar1=ncol(2, b),
                    scalar2=None, op0=mybir.AluOpType.mult)
                t2 = sb.tile([P, 512], F32, tag="t2", bufs=2)
                nc.any.tensor_tensor(
                    out=t2[:, :Do], in0=t1[:, :Do], in1=bt_sb[l][:],
                    op=mybir.AluOpType.add)
                if l < 3:
                    wo = DOUT[l]
                    o_t = sb.tile([P, 256], BF16, tag="lo", bufs=2)
                    nc.scalar.activation(
                        o_t[:, :wo], t2[:, :wo],
                        mybir.ActivationFunctionType.Relu,
                        scale=ncol(3, b))
                    wkey = (("y0", 256) if wo >= 256
                            else ("y0", 128, (l + 1) % 2))
                    wclass = 256 if wo >= 256 else 128
                    sh = shards[wkey][half]
                    nc.sync.dma_start(out=sh[off : off + rows, :],
                                      in_=o_t[:rows, :wo])
                    if b == HA_BLK - 1:
                        nc.gpsimd.collective_compute(
                            "AllGather", mybir.AluOpType.bypass,
                            replica_groups=RG,
                            ins=[shards[wkey][0][:]],
                            outs=[tabs[("y0", wclass, l + 1)][0][:]])
                    if b == NB - 1:
                        nc.gpsimd.collective_compute(
                            "AllGather", mybir.AluOpType.bypass,
                            replica_groups=RG,
                            ins=[shards[wkey][1][:]],
                            outs=[tabs[("y0", wclass, l + 1)][1][:]])
                else:
                    h = sb.tile([P, 512], BF16, tag="h", bufs=2)
                    nc.scalar.activation(
                        h[:], t2[:], mybir.ActivationFunctionType.Relu)
                    gs = sb.tile([P, P], BF16, tag="gs", bufs=2)
                    nc.vector.tensor_scalar(
                        out=gs[:], in0=iota16_sb[:],
                        scalar1=gsel_sb[:, b : b + 1], scalar2=None,
                        op0=mybir.AluOpType.is_equal)
                    nc.tensor.matmul(
                        out=psum_r[:], lhsT=gs[:], rhs=h[:],
                        start=(b == 0), stop=(b == NB - 1))

            # ---------- layers ----------
            for l in range(4):
                if stopped[0]:
                    break
                width = DIN[l]
                SS = S["S128"] if width == 128 else S["S256"]
                oh_d = oh1_d if width == 128 else oh2_d
                oh_nol = NO1L if width == 128 else NO2L
                if l == 3:
                    # swap in the 256-class indices
                    nc.sync.dma_start(out=idx_sb[:, : S2["TOT"] // 16],
                                      in_=idx2_in[:, :])
                scatter_unit(SS, tabs[("y0", width, l)], width, oh_d,
                             oh_nol,
                             lambda b, ps, wd=width, ll=l:
                                 evac_a(b, ps, wd, ll),
                             f"u{l}A")
                if KSTAGE == 10 + l:
                    dump_and_stop(tabs[("y1", width, l)][0])
                    stopped[0] = True
                    break
                scatter_unit(SS, tabs[("y1", width, l)], width, oh_d,
                             oh_nol,
                             lambda b, ps, ll=l: evac_b(b, ps, ll),
                             f"u{l}B")
                if KSTAGE == 20 + l:
                    if l < 3:
                        dump_and_stop(
                            tabs[("y0", 256 if DOUT[l] >= 256 else 128,
                                  l + 1)][0])
                    stopped[0] = True
                    break

            # ---------- readout + MLP ----------
            if not stopped[0]:
                hgp = sb.tile([P, 512], F32, tag="hgp")
                nc.any.tensor_copy(out=hgp[:], in_=psum_r[:])
                part_d = dram.tile([P, 512], F32, tag="part", name="part")
                tot_d = dram.tile([P, 512], F32, tag="tot", name="tot",
                                  addr_space="Shared")
                nc.sync.dma_start(out=part_d[:], in_=hgp[:])
                nc.gpsimd.collective_compute(
                    "AllReduce", mybir.AluOpType.add,
                    replica_groups=RG, ins=[part_d[:]], outs=[tot_d[:]])
                hg = sb.tile([P, 512], F32, tag="hg")
                nc.sync.dma_start(out=hg[:], in_=tot_d[:])

                def mm_block(x_sb, wtile, dout, bias_tile, relu):
                    xts = []
                    for kc in range(4):
                        tp = tpp.tile([P, P], F32, tag="tpf", bufs=1)
                        nc.tensor.transpose(
                            out=tp[:], in_=x_sb[:, kc * P : (kc + 1) * P],
                            identity=idf_sb[:])
                        xt = sb.tile([P, P], BF16, tag="yt", bufs=8, name="xt")
                        nc.any.tensor_copy(out=xt[:], in_=tp[:])
                        xts.append(xt)
                    ph = pp.tile([P, 512], F32, tag="dh")
                    for kc in range(4):
                        nc.tensor.matmul(
                            out=ph[:, :dout], lhsT=xts[kc][:],
                            rhs=wtile[:, kc * dout : (kc + 1) * dout],
                            start=(kc == 0), stop=(kc == 3))
                    o_t = sb.tile([P, dout], F32, tag=f"mo{dout}")
                    nc.any.tensor_tensor(
                        out=o_t[:], in0=ph[:, :dout], in1=bias_tile[:],
                        op=mybir.AluOpType.add)
                    if relu:
                        r = sb.tile([P, dout], F32, tag=f"mr{dout}")
                        nc.scalar.activation(
                            r[:], o_t[:], mybir.ActivationFunctionType.Relu)
                        return r
                    return o_t

                m1 = mm_block(hg, wm1_sb, 512, bm1_sb, relu=True)
                m2 = mm_block(m1, wm2_sb, 16, bm2_sb, relu=False)
                nc.sync.dma_start(out=out[:, :], in_=m2[:])

    nc.finalize()
    return nc


_CACHE = {}
G_OVERRIDE = None
TRACE = False
LAST_RESULTS = None


def kernel(signal, W0, b0, W1, b1, W2, b2, W3, b3, Wm1, bm1, Wm2, bm2,
           src, dst, graph_ids):
    global LAST_RESULTS
    import ml_dtypes

    signal = np.ascontiguousarray(np.asarray(signal, dtype=np.float32))
    src = np.asarray(src).astype(np.int64)
    dst = np.asarray(dst).astype(np.int64)
    graph_ids = np.asarray(graph_ids).astype(np.int64)
    N = signal.shape[0]
    G = G_OVERRIDE or 128

    key = (N, src.shape[0], G, hash(src.tobytes()) ^ hash(dst.tobytes())
           ^ hash(graph_ids.tobytes()))
    if key in _CACHE:
        S, nc = _CACHE[key]
    else:
        S = _preprocess(src, dst, graph_ids, N, G)
        nc = _build(S)
        _CACHE.clear()
        _CACHE[key] = (S, nc)

    S1, S2 = S["S128"], S["S256"]
    y0 = signal * S["norm"][:, None]
    iota_np = np.broadcast_to(np.arange(P, dtype=np.float32)[None, :],
                              (P, P)).copy()
    ident_np = np.eye(P, dtype=np.float32)
    ws = [np.asarray(w, dtype=np.float32).astype(ml_dtypes.bfloat16)
          for w in (W0, W1, W2, W3)]
    bts = [np.broadcast_to(np.asarray(b, np.float32)[None, :],
                           (P, len(b))).copy() for b in (b0, b1, b2, b3)]
    wm2_p = np.zeros((512, 16), np.float32)
    wm2_p[:, :10] = np.asarray(Wm2, dtype=np.float32)
    bm2_p = np.zeros((P, 16), np.float32)
    bm2_p[:, :10] = np.asarray(bm2, dtype=np.float32)[None, :]
    bm1_t = np.broadcast_to(np.asarray(bm1, np.float32)[None, :],
                            (P, 512)).copy()

    in_maps = []
    for c in range(NCORES):
        lo = c * NLOC
        m = {
            "sigA": y0[lo : lo + HA].astype(ml_dtypes.bfloat16),
            "sigB": y0[lo + HA : lo + NLOC].astype(ml_dtypes.bfloat16),
            "idx128": S1["idx16"][c],
            "idx256": S2["idx16"][c],
            "dsel128": S1["dsel"][c],
            "dsel256": S2["dsel"][c],
            "normc": S["normc"][c],
            "gsel": S["gsel"][c],
            "iota16": iota_np.astype(ml_dtypes.bfloat16),
            "iotaf": iota_np,
            "id16": ident_np.astype(ml_dtypes.bfloat16),
            "idf": ident_np,
            "Wm1": np.asarray(Wm1, np.float32).astype(ml_dtypes.bfloat16),
            "Bm1": bm1_t,
            "Wm2": wm2_p.astype(ml_dtypes.bfloat16),
            "Bm2": bm2_p,
        }
        for l in range(4):
            m[f"W{l}"] = ws[l]
            m[f"Bt{l}"] = bts[l]
        in_maps.append(m)

    res = run_bass_kernel_spmd(
        nc, in_maps, core_ids=list(range(NCORES)), trace=TRACE
    )
    LAST_RESULTS = res
    return np.asarray(res.results[0]["out"][:G, :10])
